# revision 110
# baseline (speedup 1.0000x reference)
"""MFGCGRU (graph-conv GRU cell) Trainium2 kernel.

Strategy: data-parallel over batch B=32 across 8 NeuronCores (4 batches
per core), NxN supports replicated. The diffusion conv is kernel-first:
S_m @ (X @ k_m), with the node contractions run as fp8e4m3 DoubleRow
matmuls (2 K-blocks per instruction at 0.5 cycles/row = 4x bf16 MAC
throughput). fp8's narrow exponent range is handled by host-side
power-of-two scaling:

  - adjacency S^T stored fp8 at x64,
  - Y = X @ (8 x kernel) quantized to fp8 (so adj-terms come out x512),
  - identity-path kernels kk0/kc0 stored bf16 at x512,
  - the attention support stays raw in fp8 (e = exp(64*QK/8) written by
    ACT straight off the QK PSUM, itself an fp8 DoubleRow matmul over
    u-halves). Its normalizer rdbc = 64/(s + rowsum(e)) is produced as a
    full [128, n] broadcast by a DoubleRow colsum against a constant 1/64
    tile, with the learned sentinel s folded in as a rank-1 matmul of a
    single fp8 row against one ones8 row; each gate group contracts e
    FIRST into its PSUM bank, multiplies the partial by rdbc on DVE, then
    accumulates identity + adjacency terms on top (x8 y-scale x64
    adj-scale = x512 everywhere).
  - gates read PSUM directly: both sigmoids are evaluated as
    0.5 + 0.5*tanh(z/2) so Relu/Exp/Tanh/Copy all live in one ACT
    function table (one LoadActFuncSet, pre-warmed at t~0 by a dummy
    exp); the 0.5s fold into the c-kernels' h-rows and the GRU tail's
    fused scalar_tensor_tensor ops.

The attention prelude (Q/K/sentinel) runs in bf16 (fp32 matmuls cost 4
cycles/row on PE vs 1 for bf16; Q/K are quantized to fp8 anyway so bf16
costs nothing numerically). All small weights ship in one packed DMA
(wpack/bpack) because each dma_start costs ~650ns of sequencer issue +
~630ns of HWDGE time; the three front loads issue on the SP, ACT and
Pool(SWDGE) queues in parallel. The prologue is built around tile-0's
serial exp chain on ACT: the eight QK matmuls run back-to-back first,
the 16 tile-0 QK-block matmuls ping-pong the two pse banks with the
colsum chasing each exp pair, and batch 0-2 Y-generation fills the PE
stream (its PSUM scratch rotates over psacc+pscr, six banks deep, so
lagging drains never head-block the in-order stream). Batch-3 Y and the
tile-0 e-contractions chase the exp stream into tile 0. Later tiles
batch their exps in pairs and pre-stream the next tile's QK + colsum.
At the phase boundary, yc[0] generation weaves in right behind the last
tile's r-updates, and phase-2 tile 0 runs pair-0's whole contraction
(only needs yc[0]) as the dense stream while yc[1] generates behind it.

e (4.2MB) and both adjacency operands (8.4MB) stay resident in SBUF so
exp and the adjacency DMAs run once across both passes. PSUM->SBUF
evacuations live on DVE and ACT only (GPSIMD cannot touch PSUM on real
hardware, whatever the cost model thinks); the SBUF-only elementwise
work (r*h, GRU tail subtract) runs on the otherwise idle Pool engine.
"""

import contextlib
import os

import numpy as np
import ml_dtypes

import concourse.bass as bass
import concourse.bacc as bacc
import concourse.tile as tile
from concourse import mybir
from concourse.bass_utils import run_bass_kernel_spmd

F32 = mybir.dt.float32
BF16 = mybir.dt.bfloat16
FP8 = mybir.dt.float8e4
AF = mybir.ActivationFunctionType
DR = mybir.MatmulPerfMode.DoubleRow

B, N, DIN, U, FD, SD = 32, 2048, 2, 64, 32, 64
NCORES = 8
BL = B // NCORES          # batches per core
NTW = 512                 # n-tile width
NT = N // NTW             # 4 n-tiles
NBW = 128                 # node-block width
NB = N // NBW             # 16 node blocks
NP = NB // 2              # 8 node-block pairs (DoubleRow)
FROWS = DIN + U           # 66

SC_ADJ = 64.0             # host scale on adjacency (fp8)
SC_Y = 8.0                # host scale on y kernels (fp8 y tiles)
SC_ID = 512.0             # host scale on identity kernels (bf16)
GATE_SCALE = 0.25 / 512.0 # sigmoid/tanh pre-scale: mean over 4 supports / 512


def _build_program():
    nc = bacc.Bacc("TRN2", debug=False, num_devices=NCORES)

    d = {}

    def din(name, shape, dt):
        d[name] = nc.dram_tensor(name, shape, dt, kind="ExternalInput").ap()

    din("xT", [BL, FROWS, N], BF16)
    din("hT", [BL, U, N], F32)
    din("a1T", [N, N], FP8)
    din("a2T", [N, N], FP8)
    din("fsT", [FD + SD, N], BF16)
    # all small bf16 weights packed column-wise into one DMA:
    # wq 0:64 | wk 64:128 | ws1 128:192 | kkall 192:576 | kk0 576:704 |
    # kcall 704:896 | kc0 896:960 | ws2 960:961
    din("wpack", [FD + SD, 961], BF16)
    # f32 bias columns: bruh | bc2 | bs1v | bs2v
    din("bpack", [2 * U, 4], F32)
    din("ones8", [NBW, 2, NBW], FP8)        # constant 1/64
    out_h = nc.dram_tensor("out", [BL, U, N], F32, kind="ExternalOutput").ap()
    uscr = nc.dram_tensor("uscr", [BL, U, N], F32).ap()

    with tile.TileContext(nc) as tc, \
            nc.allow_low_precision(reason="fp8 support contraction by design"):
        _emit(tc, d, out_h, uscr)
    nc.compile()
    return nc


def _emit(tc, d, out_h, uscr):
    nc = tc.nc
    ctx = contextlib.ExitStack()
    const = ctx.enter_context(tc.tile_pool(name="const", bufs=1))
    persist = ctx.enter_context(tc.tile_pool(name="persist", bufs=1))
    ypool = ctx.enter_context(tc.tile_pool(name="ypool", bufs=1))
    stage = ctx.enter_context(tc.tile_pool(name="stage", bufs=2))
    p3p = ctx.enter_context(tc.tile_pool(name="p3p", bufs=3))
    # PSUM budget is 8 banks (16KB/partition):
    #   psacc: 3 x 1 bank  (gate accumulators)
    #   pse:   1 x 2 banks (QK-pair scratch in phase 1, spare acc in phase 2)
    #   pscr:  "scr" 3 x 1 bank (y/yc/prelude/colsum scratch)
    psacc = ctx.enter_context(tc.tile_pool(name="psacc", bufs=3, space="PSUM"))
    pse = ctx.enter_context(tc.tile_pool(name="pse", bufs=1, space="PSUM"))
    pscr = ctx.enter_context(tc.tile_pool(name="pscr", bufs=3, space="PSUM"))

    # ---- constants / weights in SBUF ----
    def cload(name):
        ap = d[name]
        t = const.tile(list(ap.shape), ap.dtype, name=f"c_{name}")
        nc.sync.dma_start(out=t, in_=ap)
        return t

    # DMA order = earliest-consumer order, with the two PE-unblocking
    # tensors (fsT's feat rows for the QK prelude, xT0 for y-generation)
    # chunked so the first matmuls start early. Small weights ride in one
    # packed DMA (each separate DMA costs ~700ns of queue time).
    fsT = const.tile([FD + SD, N], BF16, name="c_fsT")
    xT = [persist.tile([FROWS, N], BF16, name=f"xT{b}", tag=f"xT{b}")
          for b in range(BL)]
    # the PE-unblocking loads issue on three different queues so their
    # HWDGE descriptor slots pack back-to-back from t=0; the ACT queue has
    # no preamble, so the first QK tile's feat rows land there first
    wpack = const.tile([FD + SD, 961], BF16, name="c_wpack")
    nc.scalar.dma_start(out=fsT[0:FD, 0:NTW], in_=d["fsT"][0:FD, 0:NTW])
    nc.sync.dma_start(out=wpack, in_=d["wpack"])
    nc.scalar.dma_start(out=fsT[0:FD, NTW:], in_=d["fsT"][0:FD, NTW:])
    nc.gpsimd.dma_start(out=xT[0], in_=d["xT"][0])
    bpack = cload("bpack")
    ones8 = cload("ones8")
    wq = wpack[0:FD, 0:64]
    wk = wpack[0:FD, 64:128]
    ws1 = wpack[:, 128:192]
    kkall = wpack[0:FROWS, 192:576]
    kk0 = wpack[0:FROWS, 576:704]
    kcall = wpack[0:FROWS, 704:896]
    kc0 = wpack[0:FROWS, 896:960]
    ws2 = wpack[0:U, 960:961]
    bruh = bpack[:, 0:1]
    bc2 = bpack[:, 1:2]
    bs1v = bpack[0:U, 2:3]
    bs2v = bpack[0:1, 3:4]
    nc.sync.dma_start(out=xT[1], in_=d["xT"][1])
    nc.sync.dma_start(out=fsT[FD:, :], in_=d["fsT"][FD:, :])
    _adj_cache = {}

    def adjslice(name, t):
        # persist: each slice is DMA'd once (phase 1) and reused in phase 2
        if (name, t) not in _adj_cache:
            sl = d[name][:, t * NTW:(t + 1) * NTW]
            a = persist.tile([NBW, NB, NTW], FP8, name=f"{name}_{t}",
                             tag=f"{name}_{t}")
            nc.sync.dma_start(out=a,
                              in_=sl.rearrange("(j p) w -> p j w", p=NBW))
            _adj_cache[(name, t)] = a
        return _adj_cache[(name, t)]

    adjslice("a1T", 0)
    nc.sync.dma_start(out=xT[2], in_=d["xT"][2])
    adjslice("a2T", 0)
    nc.sync.dma_start(out=xT[3], in_=d["xT"][3])
    # sentinel row: folded into the colsum as a rank-1 matmul against a
    # single ones8 row, so no zero-filled [128, N] tile (or its memset)
    s8row = const.tile([1, N], FP8, name="s8row")
    one_bc = const.tile([128, NTW], F32, name="one_bc")
    nc.vector.memset(one_bc, 1.0)
    # preload the exp_and_others ACT table at t~0 (a real activation would
    # otherwise pay the 1.3us table load right when QT evacuations start)
    actwarm = const.tile([1, 1], F32, name="actwarm")
    nc.scalar.activation(actwarm, one_bc[0:1, 0:1], AF.Exp)

    QT = persist.tile([U // 2, 2, N], FP8, name="QT", tag="QT")
    KT = persist.tile([U // 2, 2, N], FP8, name="KT", tag="KT")
    # resident raw attention support e^T = exp(KQ^T/8), fp8
    et = [persist.tile([NBW, NB, NTW], FP8, name=f"et{t}", tag=f"et{t}")
          for t in range(NT)]
    # rdbc[t][p, n] = 64/d[n]: e-term normalizer, applied to PSUM e-partials
    rdbc = [persist.tile([NBW, NTW], F32, name=f"rdbc{t}", tag=f"rdbc{t}")
            for t in range(NT)]

    # ---- prelude thunks: Q^T, K^T (critical: gate eg0) and the sentinel
    # s chain (only needed by the end of each colsum). QK evacuations split
    # DVE/ACT; the sentinel chain evacuates on the otherwise-idle Pool. ----
    def qk_thunks(t):
        sl = slice(t * NTW, (t + 1) * NTW)

        def t_pk():
            pk = pscr.tile([U, NTW], F32, name="pk", tag="scr")
            nc.tensor.matmul(pk, wk, fsT[0:FD, sl], start=True, stop=True)
            nc.vector.tensor_scalar(KT[:, 0, sl], pk[0:U // 2, :], 8.0, 0.0,
                                    mybir.AluOpType.mult, mybir.AluOpType.max)
            nc.scalar.activation(KT[:, 1, sl], pk[U // 2:U, :], AF.Relu,
                                 scale=8.0)

        def t_pq():
            pq = pscr.tile([U, NTW], F32, name="pq", tag="scr")
            nc.tensor.matmul(pq, wq, fsT[0:FD, sl], start=True, stop=True)
            nc.vector.tensor_scalar(QT[:, 0, sl], pq[0:U // 2, :], 8.0, 0.0,
                                    mybir.AluOpType.mult, mybir.AluOpType.max)
            nc.scalar.activation(QT[:, 1, sl], pq[U // 2:U, :], AF.Relu,
                                 scale=8.0)

        return [t_pk, t_pq]

    def s_thunks(t):
        sl = slice(t * NTW, (t + 1) * NTW)
        s1t = stage.tile([U, NTW], BF16, name="s1t", tag="sig")

        def t_ps1():
            ps1 = pscr.tile([U, NTW], F32, name="ps1", tag="scr")
            nc.tensor.matmul(ps1, ws1, fsT[:, sl], start=True, stop=True)
            if t == 0:
                nc.vector.tensor_scalar(s1t, ps1, bs1v, 0.0,
                                        mybir.AluOpType.add,
                                        mybir.AluOpType.max)
            else:
                nc.scalar.activation(s1t, ps1, AF.Relu, bias=bs1v)

        def t_ps2():
            ps2 = pscr.tile([1, NTW], F32, name="ps2", tag="scr")
            nc.tensor.matmul(ps2, ws2, s1t, start=True, stop=True)
            if t == 0:
                nc.vector.tensor_scalar(s8row[0:1, sl], ps2, bs2v, 0.0,
                                        mybir.AluOpType.add,
                                        mybir.AluOpType.max)
            else:
                nc.scalar.activation(s8row[0:1, sl], ps2, AF.Relu,
                                     bias=bs2v)

        return [t_ps1, t_ps2]

    # ---- tile-0 e-generation: singles on one rotating bank, each QK
    # matmul evacuated by its own exp. The 16 serial exps are tile-0's
    # critical path; the PE stream runs y-generation between them. ----
    def eg0_thunks():
        sl = slice(0, NTW)
        eg0 = pse.tile([NBW, 2, NTW], F32, name="eg0", tag="pse")

        def mk(j):
            def f():
                pej = eg0[:, j % 2, :]
                nc.tensor.matmul(pej, KT[:, :, j * NBW:(j + 1) * NBW],
                                 QT[:, :, sl], start=True, stop=True,
                                 perf_mode=DR)
                nc.scalar.activation(et[0][:, j, :], pej, AF.Exp,
                                     scale=0.125 / 64.0)
            return f
        return [mk(j) for j in range(NB)]

    # ---- e-generation for tiles 1..3: pairs with one batched exp ----
    def eg_pair_thunks(t):
        sl = slice(t * NTW, (t + 1) * NTW)

        def mk(j):
            def f():
                pp = pse.tile([NBW, 2, NTW], F32, name="pp", tag="pse")
                nc.tensor.matmul(pp[:, 0, :], KT[:, :, j * NBW:(j + 1) * NBW],
                                 QT[:, :, sl], start=True, stop=True,
                                 perf_mode=DR)
                nc.tensor.matmul(pp[:, 1, :],
                                 KT[:, :, (j + 1) * NBW:(j + 2) * NBW],
                                 QT[:, :, sl], start=True, stop=True,
                                 perf_mode=DR)
                nc.scalar.activation(et[t][:, j:j + 2, :], pp, AF.Exp,
                                     scale=0.125 / 64.0)
            return f
        return [mk(j) for j in range(0, NB, 2)]

    def colsum_thunks(t):
        """rdbc[t][p, n] = 64 / (s[n] + colsum(e^T)[n]), all partitions."""
        sl = slice(t * NTW, (t + 1) * NTW)
        pcs = pscr.tile([NBW, NTW], F32, name="pcs", tag="scr")
        th = []
        for k in range(NP):
            def f(k=k):
                nc.tensor.matmul(pcs, ones8, et[t][:, 2 * k:2 * k + 2, :],
                                 start=(k == 0), stop=False, perf_mode=DR)
            th.append(f)
        th.append(lambda: nc.tensor.matmul(
            pcs, ones8[0:1, 0, :], s8row[0:1, sl], start=False, stop=True))
        th.append(lambda: nc.vector.reciprocal(rdbc[t], pcs))
        return th

    # ---- phase-1 Y tiles: Y[m,b] = X_b @ [8*k_r[m]|8*k_u[m]] ----
    # stored fp8 [node%128, node//128, m, u']; copies split DVE/ACT
    y = [ypool.tile([NBW, NB, 3, 2 * U], FP8, name=f"y_{b}", tag=f"y{b}")
         for b in range(BL)]

    # drain-engine assignment per batch (even-j, odd-j): chosen so each
    # prologue engine's queue stays under the ~17us exp-bound window, and
    # ACT only gets drains whose emission lands after the exp stream.
    Y_ENG = {0: ("dve", "dve"), 1: ("dve", "act"),
             2: ("dve", "dve"), 3: ("dve", "act")}

    def ygen_thunks(b):
        # batches 0-2 run in the prologue, where the psacc banks are still
        # idle: alternating psacc/pscr gives a 6-deep scratch rotation so
        # lagging drains never head-block the PE stream.
        def mk(j):
            def f():
                nsl = slice(j * NBW, (j + 1) * NBW)
                if b < 3 and j % 2 == 1:
                    py = psacc.tile([NBW, 3 * 2 * U], F32, name="py",
                                    tag="acc")
                else:
                    py = pscr.tile([NBW, 3 * 2 * U], F32, name="py",
                                   tag="scr")
                nc.tensor.matmul(py, xT[b][:, nsl], kkall, start=True,
                                 stop=True)
                eng = Y_ENG[b][j % 2]
                src = py.rearrange("p (m u) -> p m u", m=3)
                if eng == "dve":
                    nc.vector.tensor_copy(y[b][:, j, :, :], src)
                else:
                    nc.scalar.activation(
                        y[b][:, j, :, :].rearrange("p m u -> p (m u)"),
                        py, AF.Copy)
            return f
        return [mk(j) for j in range(NB)]

    def interleave(main, extra, ratio=2):
        mi = ei = 0
        while mi < len(main) or ei < len(extra):
            for _ in range(ratio):
                if mi < len(main):
                    main[mi](); mi += 1
            if ei < len(extra):
                extra[ei](); ei += 1

    # =================== phase 1: r & u gates ===================
    # Each gate group: e-support contraction first (own PSUM group), then a
    # DVE multiply by rdbc[t] on the PSUM partial, then identity + adjacency
    # terms accumulate on top (start=False).
    def e_thunks(yt, t, pa):
        th = []
        for k in range(NP):
            def f(k=k):
                nc.tensor.matmul(pa, yt[:, 2 * k:2 * k + 2, 2, :],
                                 et[t][:, 2 * k:2 * k + 2, :],
                                 start=(k == 0), stop=(k == NP - 1),
                                 perf_mode=DR)
            th.append(f)
        return th

    def rest_thunks1(b, t, sl, a1, a2, pa):
        th = [lambda: nc.vector.tensor_mul(pa, pa, rdbc[t]),
              lambda: nc.tensor.matmul(pa, kk0, xT[b][:, sl], start=False,
                                       stop=False, skip_group_check=True)]
        for m, mov in ((0, a1), (1, a2)):
            for k in range(NP):
                def f(m=m, mov=mov, k=k):
                    nc.tensor.matmul(pa, y[b][:, 2 * k:2 * k + 2, m, :],
                                     mov[:, 2 * k:2 * k + 2, :],
                                     start=False,
                                     stop=(m == 1 and k == NP - 1),
                                     perf_mode=DR, skip_group_check=True)
                th.append(f)
        return th

    def gate1(b, t, sl, pa):
        def f():
            # sigmoid(z) computed as 0.5 + 0.5*tanh(z/2): keeps every ACT
            # function in the exp_and_others table (no table reloads).
            # sig holds T = tanh(z/2) = 2*sigmoid(z) - 1.
            sig = stage.tile([128, NTW], F32, name="sig", tag="sig")
            nc.scalar.activation(sig, pa, AF.Tanh, scale=GATE_SCALE / 2,
                                 bias=bruh)
            # rows 0:64 in place: (T_r + 1)*h = 2*r*h; the c-pass kernels'
            # h-rows carry a compensating 1/2. u kept as raw T_u.
            rtmp = stage.tile([U, NTW], F32, name="rtmp", tag="rtmp")
            nc.gpsimd.tensor_add(rtmp, sig[0:U, :], one_bc[0:U, :])
            nc.gpsimd.tensor_mul(xT[b][0:U, sl], rtmp, xT[b][0:U, sl])
            nc.sync.dma_start(out=uscr[b][:, sl], in_=sig[U:128, :])
        return [f]

    yc = [None, None]

    def ycgen_half(yct, p, half):
        b = 2 * p + half
        usl = slice(half * U, (half + 1) * U)
        th = []
        for j in range(NB):
            def f(b=b, usl=usl, j=j, yct=yct, half=half):
                nsl = slice(j * NBW, (j + 1) * NBW)
                pyc = pscr.tile([NBW, 3 * U], F32, name="pyc", tag="scr")
                nc.tensor.matmul(pyc, xT[b][:, nsl], kcall,
                                 start=True, stop=True)
                dst = yct[:, j, :, usl]
                src = pyc.rearrange("p (m u) -> p m u", m=3)
                if (half * NB + j) % 2 == 0:
                    nc.vector.tensor_copy(dst, src)
                else:
                    nc.scalar.activation(dst, src, AF.Copy)
            th.append(f)
        return th

    def ycgen_thunks(yct, p):
        return ycgen_half(yct, p, 0) + ycgen_half(yct, p, 1)

    # ---------- prologue ----------
    # Each QK-prelude tile is chased immediately by its four tile-0 QK
    # blocks (subtile deps let eg0[j] start as soon as its K block lands),
    # with the colsum chasing each exp pair and y-generation as PE filler.
    # The serial exp stream on ACT is tile-0's critical path; everything
    # else here is arranged to keep it back-to-back.
    qk = []
    for t in range(NT):
        qk += qk_thunks(t)
    interleave(qk, ygen_thunks(0), ratio=1)
    eg0 = eg0_thunks()
    cs0 = colsum_thunks(0)
    crit = []
    for k in range(NP):
        crit += [eg0[2 * k], eg0[2 * k + 1], cs0[k]]
    crit += cs0[NP:]
    sth = []
    for t in range(NT):
        sth += s_thunks(t)
    filler = []
    yg1 = ygen_thunks(1)
    for i, f in enumerate(yg1):
        filler.append(f)
        if i % 2 == 1 and sth:
            filler.append(sth.pop(0))
    filler += sth
    interleave(filler, crit, ratio=2)
    # tile-1's QK pairs ride the late prologue (their pse slots free as
    # tile-0's exps complete; their exps follow et0's on ACT) so rdbc[1]
    # is ready well before tile 1 needs it
    interleave(ygen_thunks(2), eg_pair_thunks(1), ratio=2)

    for t in range(NT):
        sl = slice(t * NTW, (t + 1) * NTW)
        a1 = adjslice("a1T", t)
        a2 = adjslice("a2T", t)
        pa = [psacc.tile([128, NTW], F32, name="pa", tag="acc")
              for _ in range(BL)]
        # schedule: e-streams lead their group's rest by one slot so the
        # DVE rdbc-multiply hides under the next group's e-stream.
        main = []
        main += e_thunks(y[0], t, pa[0]) + e_thunks(y[1], t, pa[1])
        if t == 0:
            # batch-3 y-generation woven into the rdbc-gated stretch
            m1 = e_thunks(y[2], t, pa[2]) \
                + rest_thunks1(0, t, sl, a1, a2, pa[0]) \
                + gate1(0, t, sl, pa[0])
            mi = []
            yg3 = ygen_thunks(3)
            k = 0
            for i, f in enumerate(m1):
                mi.append(f)
                while k < len(yg3) * (i + 1) // len(m1):
                    mi.append(yg3[k]); k += 1
            mi.extend(yg3[k:])
            main += mi
        else:
            main += e_thunks(y[2], t, pa[2])
            main += rest_thunks1(0, t, sl, a1, a2, pa[0]) \
                + gate1(0, t, sl, pa[0])
        m2 = e_thunks(y[3], t, pa[3]) \
            + rest_thunks1(1, t, sl, a1, a2, pa[1]) + gate1(1, t, sl, pa[1])
        post = []
        post += rest_thunks1(2, t, sl, a1, a2, pa[2]) + gate1(2, t, sl, pa[2])
        post += rest_thunks1(3, t, sl, a1, a2, pa[3]) + gate1(3, t, sl, pa[3])
        extra = eg_pair_thunks(t + 1) if 0 < t < NT - 1 else []
        if t == NT - 1:
            # yc[0] reads post-rh xT rows: batch 0's half can weave as soon
            # as gate1(0) has run, batch 1's into the post stream.
            yc[0] = ypool.tile([NBW, NB, 3, 2 * U], FP8, name="yc_0",
                               tag="y0")
            interleave(main, [])
            interleave(m2, ycgen_half(yc[0], 0, 0), ratio=1)
            interleave(post, ycgen_half(yc[0], 0, 1), ratio=1)
        else:
            allm = main + m2 + post
            alle = extra
            interleave(allm, alle, ratio=max(1, len(allm) // max(1, len(alle))))
        # colsum for t+1 at tile end: its exp deps had the whole tile to run
        if t + 1 < NT:
            for f in colsum_thunks(t + 1):
                f()

    # =================== phase 2+3: c gate & h_new ===================
    yc[1] = ypool.tile([NBW, NB, 3, 2 * U], FP8, name="yc_1", tag="y1")
    a1 = adjslice("a1T", 0)
    a2 = adjslice("a2T", 0)

    def hu_prefetch(p, t, sl, hu):
        # single 2-batch DMA each for h and u: [2, 64, w] -> [128, w]
        b0 = 2 * p

        def f():
            hp = p3p.tile([128, NTW], F32, name="hp", tag="hp")
            up = p3p.tile([128, NTW], F32, name="up", tag="up")
            nc.sync.dma_start(
                out=hp, in_=d["hT"][b0:b0 + 2, :, sl].rearrange(
                    "b u w -> (b u) w"))
            nc.sync.dma_start(
                out=up, in_=uscr[b0:b0 + 2, :, sl].rearrange(
                    "b u w -> (b u) w"))
            hu.extend((hp, up))
        return [f]

    def rest_thunks2(p, t, sl, a1, a2, pa, hu):
        b0, b1 = 2 * p, 2 * p + 1

        th = [lambda: nc.vector.tensor_mul(pa, pa, rdbc[t]),
              lambda: nc.tensor.matmul(pa[0:U, :], kc0, xT[b0][:, sl],
                                       start=False, stop=False,
                                       skip_group_check=True),
              lambda: nc.tensor.matmul(pa[U:128, :], kc0, xT[b1][:, sl],
                                       start=False, stop=False,
                                       skip_group_check=True)]
        for m, mov in ((0, a1), (1, a2)):
            for k in range(NP):
                def f(m=m, mov=mov, k=k):
                    nc.tensor.matmul(pa, yc[p][:, 2 * k:2 * k + 2, m, :],
                                     mov[:, 2 * k:2 * k + 2, :],
                                     start=False,
                                     stop=(m == 1 and k == NP - 1),
                                     perf_mode=DR, skip_group_check=True)
                th.append(f)
        return th

    def tail2(p, t, sl, pa, hu, splits=2):
        b0, b1 = 2 * p, 2 * p + 1

        def f():
            hp, up = hu
            ct = stage.tile([128, NTW], F32, name="ct", tag="sig")
            t1 = p3p.tile([128, NTW], F32, name="t1", tag="t1")
            # compute in column chunks so ACT/Pool/DVE stages pipeline, but
            # write out full-width (HWDGE descriptor slots are the scarce
            # resource at the drain)
            for c0 in range(0, NTW, NTW // splits):
                cs = slice(c0, c0 + NTW // splits)
                nc.scalar.activation(ct[:, cs], pa[:, cs], AF.Tanh,
                                     scale=GATE_SCALE, bias=bc2)
                # h_new = c + u*(h-c) with up = T_u = 2u-1:
                # t1 = h-c (Pool); t1 = (T_u+1)*t1; t1 = t1/2 + c (DVE)
                nc.gpsimd.tensor_sub(t1[:, cs], hp[:, cs], ct[:, cs])
                nc.vector.scalar_tensor_tensor(
                    t1[:, cs], up[:, cs], 1.0, t1[:, cs],
                    mybir.AluOpType.add, mybir.AluOpType.mult)
                nc.vector.scalar_tensor_tensor(
                    t1[:, cs], t1[:, cs], 0.5, ct[:, cs],
                    mybir.AluOpType.mult, mybir.AluOpType.add)
                if splits == 4 and c0 + NTW // splits in (NTW // 2, NTW):
                    hs = slice(c0 + NTW // splits - NTW // 2,
                               c0 + NTW // splits)
                    # the two final half-writes issue on different queues
                    # so their HWDGE/issue pipelines overlap
                    eng = nc.sync if hs.start == 0 else nc.scalar
                    eng.dma_start(
                        out=out_h[b0:b0 + 2, :,
                                  t * NTW + hs.start:t * NTW + hs.stop]
                        .rearrange("b u w -> (b u) w"),
                        in_=t1[:, hs])
            if splits != 4:
                nc.sync.dma_start(
                    out=out_h[b0:b0 + 2, :, sl].rearrange("b u w -> (b u) w"),
                    in_=t1)
        return [f]

    prev = []
    pa_pre = pa_pre1 = None
    for t in range(NT):
        sl = slice(t * NTW, (t + 1) * NTW)
        if t > 0:
            a1 = adjslice("a1T", t)
            a2 = adjslice("a2T", t)
        pa0 = pa_pre if pa_pre is not None else \
            psacc.tile([128, NTW], F32, name="pa2", tag="acc")
        pa1 = pa_pre1 if pa_pre1 is not None else \
            psacc.tile([128, NTW], F32, name="pa2", tag="acc")
        hu0, hu1 = [], []
        main = hu_prefetch(0, t, sl, hu0) + prev + hu_prefetch(1, t, sl, hu1)
        if t == 0:
            # p0's whole contraction only needs yc[0] (built at the end of
            # phase 1): run it as the dense stream with yc[1] generation
            # woven in as drain-paced filler, then chase e(yc[1]).
            p0w = e_thunks(yc[0], t, pa0) \
                + rest_thunks2(0, t, sl, a1, a2, pa0, hu0)
            ycg = ycgen_thunks(yc[1], 1)
            mi = []
            k = 0
            for i, f in enumerate(p0w):
                mi.append(f)
                while k < len(ycg) * (i + 1) // len(p0w):
                    mi.append(ycg[k]); k += 1
            main += mi + ycg[k:] + e_thunks(yc[1], t, pa1)
        else:
            if pa_pre is None:
                main += e_thunks(yc[0], t, pa0)
            if pa_pre1 is None:
                main += e_thunks(yc[1], t, pa1)
            main += rest_thunks2(0, t, sl, a1, a2, pa0, hu0)
        # pre-stream the next tile's e-contractions: PE filler while this
        # tile's tanh/tail chains drain (p1's bank alternates between the
        # phase-2-idle pse pool and a dedicated pscr tag)
        if t + 1 < NT:
            pa_pre = psacc.tile([128, NTW], F32, name="pa2", tag="acc")
            main += e_thunks(yc[0], t + 1, pa_pre)
            if t % 2 == 0:
                pa_pre1 = pse.tile([128, NTW], F32, name="pa2e", tag="pse")
            else:
                pa_pre1 = psacc.tile([128, NTW], F32, name="pa2e", tag="acc")
            main += e_thunks(yc[1], t + 1, pa_pre1)
        else:
            pa_pre = pa_pre1 = None
        main += tail2(0, t, sl, pa0, hu0)
        for f in main:
            f()
        # carry p=1's rest into the next tile so its DVE multiply and the
        # tail chain overlap with the next tile's e-streams
        prev = rest_thunks2(1, t, sl, a1, a2, pa1, hu1) \
            + tail2(1, t, sl, pa1, hu1, splits=4 if t == NT - 1 else 2)
    for f in prev:
        f()

    ctx.close()


_CACHE = {}


def _get_program():
    if "nc" not in _CACHE:
        _CACHE["nc"] = _build_program()
    return _CACHE["nc"]


def _prep_inputs(inputs, h_prev, adj1, adj2, feat, SE, Wq, Wk, Ws1, bs1, Ws2,
                 bs2, r_kernel, r_bias, u_kernel, u_bias, c_kernel, c_bias):
    bf = ml_dtypes.bfloat16
    f8 = ml_dtypes.float8_e4m3fn
    f32 = np.float32
    perm = list(range(DIN, FROWS)) + list(range(DIN))  # [h(64); inputs(2)]

    h3 = np.asarray(h_prev, f32).reshape(B, N, U)
    hT = np.ascontiguousarray(h3.transpose(0, 2, 1))            # [B, U, N]
    inT = np.asarray(inputs, f32).transpose(0, 2, 1)            # [B, DIN, N]
    xT = np.concatenate([hT, inT], axis=1).astype(bf)           # [B, 66, N]

    rk = np.asarray(r_kernel, f32)[:, perm, :]
    uk = np.asarray(u_kernel, f32)[:, perm, :]
    ck = np.asarray(c_kernel, f32)[:, perm, :]
    kkall = np.concatenate(
        [np.concatenate([rk[m], uk[m]], axis=1) for m in (1, 2, 3)],
        axis=1) * SC_Y                                          # [66, 384]
    kk0 = np.concatenate([rk[0], uk[0]], axis=1) * SC_ID        # [66, 128]
    # c-pass x_cat holds 2*r*h in its h-rows (r comes from the tanh-form
    # sigmoid as T_r + 1); compensate with a 1/2 on the kernels' h-rows.
    ck = ck.copy()
    ck[:, 0:U, :] *= 0.5
    kcall = np.concatenate([ck[1], ck[2], ck[3]], axis=1) * SC_Y
    kc0 = ck[0] * SC_ID

    wpk = np.zeros((FD + SD, 961), f32)
    wpk[0:FD, 0:64] = np.asarray(Wq, f32)
    wpk[0:FD, 64:128] = np.asarray(Wk, f32)
    wpk[:, 128:192] = np.asarray(Ws1, f32)
    wpk[0:FROWS, 192:576] = kkall
    wpk[0:FROWS, 576:704] = kk0
    wpk[0:FROWS, 704:896] = kcall
    wpk[0:FROWS, 896:960] = kc0
    wpk[0:U, 960] = np.asarray(Ws2, f32).reshape(U)
    bpk = np.zeros((2 * U, 4), f32)
    bpk[:, 0] = np.concatenate([np.asarray(r_bias, f32).mean(0),
                                np.asarray(u_bias, f32).mean(0)]) * 0.5
    bpk[:, 1] = np.tile(np.asarray(c_bias, f32).mean(0), 2)
    bpk[0:U, 2] = np.asarray(bs1, f32)
    bpk[0, 3] = np.asarray(bs2, f32).reshape(1)[0]

    shared = {
        "a1T": np.ascontiguousarray(np.asarray(adj1, f32).T * SC_ADJ).astype(f8),
        "a2T": np.ascontiguousarray(np.asarray(adj2, f32).T * SC_ADJ).astype(f8),
        "fsT": np.ascontiguousarray(
            np.concatenate([np.asarray(feat, f32).T, np.asarray(SE, f32).T],
                           axis=0)).astype(bf),
        "wpack": wpk.astype(bf),
        "bpack": bpk,
        "ones8": np.full((NBW, 2, NBW), 1.0 / 64.0, f8),
    }
    in_maps = []
    for c in range(NCORES):
        bsl = slice(c * BL, (c + 1) * BL)
        m = dict(shared)
        m["xT"] = np.ascontiguousarray(xT[bsl])
        m["hT"] = np.ascontiguousarray(hT[bsl])
        in_maps.append(m)
    return in_maps


def kernel(**inputs):
    os.environ.setdefault("NEURON_RT_RESET_CORES", "1")
    nc = _get_program()
    in_maps = _prep_inputs(**inputs)
    res = None
    err = None
    for _ in range(2):
        try:
            res = run_bass_kernel_spmd(nc, in_maps, list(range(NCORES)))
            break
        except Exception as e:  # e.g. a wedged device; retry once
            err = e
    if res is None:
        raise err
    outs = []
    for c in range(NCORES):
        o = res.results[c]["out"]                     # [BL, U, N] f32
        outs.append(o.transpose(0, 2, 1).reshape(BL, N * U))
    return np.concatenate(outs, axis=0).astype(np.float32)


# revision 111
# speedup vs baseline: 1.0025x; 1.0025x over previous
"""MFGCGRU (graph-conv GRU cell) Trainium2 kernel.

Strategy: data-parallel over batch B=32 across 8 NeuronCores (4 batches
per core), NxN supports replicated. The diffusion conv is kernel-first:
S_m @ (X @ k_m), with the node contractions run as fp8e4m3 DoubleRow
matmuls (2 K-blocks per instruction at 0.5 cycles/row = 4x bf16 MAC
throughput). fp8's narrow exponent range is handled by host-side
power-of-two scaling:

  - adjacency S^T stored fp8 at x64,
  - Y = X @ (8 x kernel) quantized to fp8 (so adj-terms come out x512),
  - identity-path kernels kk0/kc0 stored bf16 at x512,
  - the attention support stays raw in fp8 (e = exp(64*QK/8) written by
    ACT straight off the QK PSUM, itself an fp8 DoubleRow matmul over
    u-halves). Its normalizer rdbc = 64/(s + rowsum(e)) is produced as a
    full [128, n] broadcast by a DoubleRow colsum against a constant 1/64
    tile, with the learned sentinel s folded in as a rank-1 matmul of a
    single fp8 row against one ones8 row; each gate group contracts e
    FIRST into its PSUM bank, multiplies the partial by rdbc on DVE, then
    accumulates identity + adjacency terms on top (x8 y-scale x64
    adj-scale = x512 everywhere).
  - gates read PSUM directly: both sigmoids are evaluated as
    0.5 + 0.5*tanh(z/2) so Relu/Exp/Tanh/Copy all live in one ACT
    function table (one LoadActFuncSet, pre-warmed at t~0 by a dummy
    exp); the 0.5s fold into the c-kernels' h-rows and the GRU tail's
    fused scalar_tensor_tensor ops.

The attention prelude (Q/K/sentinel) runs in bf16 (fp32 matmuls cost 4
cycles/row on PE vs 1 for bf16; Q/K are quantized to fp8 anyway so bf16
costs nothing numerically). All small weights ship in one packed DMA
(wpack/bpack) because each dma_start costs ~650ns of sequencer issue +
~630ns of HWDGE time; the three front loads issue on the SP, ACT and
Pool(SWDGE) queues in parallel. The prologue is built around tile-0's
serial exp chain on ACT: the eight QK matmuls run back-to-back first,
the 16 tile-0 QK-block matmuls ping-pong the two pse banks with the
colsum chasing each exp pair, and batch 0-2 Y-generation fills the PE
stream (its PSUM scratch rotates over psacc+pscr, six banks deep, so
lagging drains never head-block the in-order stream). Batch-3 Y and the
tile-0 e-contractions chase the exp stream into tile 0. Later tiles
batch their exps in pairs and pre-stream the next tile's QK + colsum.
At the phase boundary, yc[0] generation weaves in right behind the last
tile's r-updates, and phase-2 tile 0 runs pair-0's whole contraction
(only needs yc[0]) as the dense stream while yc[1] generates behind it.

e (4.2MB) and both adjacency operands (8.4MB) stay resident in SBUF so
exp and the adjacency DMAs run once across both passes. PSUM->SBUF
evacuations live on DVE and ACT only (GPSIMD cannot touch PSUM on real
hardware, whatever the cost model thinks); the SBUF-only elementwise
work (r*h, GRU tail subtract) runs on the otherwise idle Pool engine.
"""

import contextlib
import os

import numpy as np
import ml_dtypes

import concourse.bass as bass
import concourse.bacc as bacc
import concourse.tile as tile
from concourse import mybir
from concourse.bass_utils import run_bass_kernel_spmd

F32 = mybir.dt.float32
BF16 = mybir.dt.bfloat16
FP8 = mybir.dt.float8e4
AF = mybir.ActivationFunctionType
DR = mybir.MatmulPerfMode.DoubleRow

B, N, DIN, U, FD, SD = 32, 2048, 2, 64, 32, 64
NCORES = 8
BL = B // NCORES          # batches per core
NTW = 512                 # n-tile width
NT = N // NTW             # 4 n-tiles
NBW = 128                 # node-block width
NB = N // NBW             # 16 node blocks
NP = NB // 2              # 8 node-block pairs (DoubleRow)
FROWS = DIN + U           # 66

SC_ADJ = 64.0             # host scale on adjacency (fp8)
SC_Y = 8.0                # host scale on y kernels (fp8 y tiles)
SC_ID = 512.0             # host scale on identity kernels (bf16)
GATE_SCALE = 0.25 / 512.0 # sigmoid/tanh pre-scale: mean over 4 supports / 512


def _build_program():
    nc = bacc.Bacc("TRN2", debug=False, num_devices=NCORES)

    d = {}

    def din(name, shape, dt):
        d[name] = nc.dram_tensor(name, shape, dt, kind="ExternalInput").ap()

    din("xT", [BL, FROWS, N], BF16)
    din("hT", [BL, U, N], F32)
    din("a1T", [N, N], FP8)
    din("a2T", [N, N], FP8)
    din("fsT", [FD + SD, N], BF16)
    # all small bf16 weights packed column-wise into one DMA:
    # wq 0:64 | wk 64:128 | ws1 128:192 | kkall 192:576 | kk0 576:704 |
    # kcall 704:896 | kc0 896:960 | ws2 960:961
    din("wpack", [FD + SD, 961], BF16)
    # f32 bias columns: bruh | bc2 | bs1v | bs2v
    din("bpack", [2 * U, 4], F32)
    din("ones8", [NBW, 2, NBW], FP8)        # constant 1/64
    out_h = nc.dram_tensor("out", [BL, U, N], F32, kind="ExternalOutput").ap()
    uscr = nc.dram_tensor("uscr", [BL, U, N], F32).ap()

    with tile.TileContext(nc) as tc, \
            nc.allow_low_precision(reason="fp8 support contraction by design"):
        _emit(tc, d, out_h, uscr)
    nc.compile()
    return nc


def _emit(tc, d, out_h, uscr):
    nc = tc.nc
    ctx = contextlib.ExitStack()
    const = ctx.enter_context(tc.tile_pool(name="const", bufs=1))
    persist = ctx.enter_context(tc.tile_pool(name="persist", bufs=1))
    ypool = ctx.enter_context(tc.tile_pool(name="ypool", bufs=1))
    stage = ctx.enter_context(tc.tile_pool(name="stage", bufs=2))
    p3p = ctx.enter_context(tc.tile_pool(name="p3p", bufs=3))
    # PSUM budget is 8 banks (16KB/partition):
    #   psacc: 3 x 1 bank  (gate accumulators)
    #   pse:   1 x 2 banks (QK-pair scratch in phase 1, spare acc in phase 2)
    #   pscr:  "scr" 3 x 1 bank (y/yc/prelude/colsum scratch)
    psacc = ctx.enter_context(tc.tile_pool(name="psacc", bufs=3, space="PSUM"))
    pse = ctx.enter_context(tc.tile_pool(name="pse", bufs=1, space="PSUM"))
    pscr = ctx.enter_context(tc.tile_pool(name="pscr", bufs=3, space="PSUM"))

    # ---- constants / weights in SBUF ----
    def cload(name):
        ap = d[name]
        t = const.tile(list(ap.shape), ap.dtype, name=f"c_{name}")
        nc.sync.dma_start(out=t, in_=ap)
        return t

    # DMA order = earliest-consumer order, with the two PE-unblocking
    # tensors (fsT's feat rows for the QK prelude, xT0 for y-generation)
    # chunked so the first matmuls start early. Small weights ride in one
    # packed DMA (each separate DMA costs ~700ns of queue time).
    fsT = const.tile([FD + SD, N], BF16, name="c_fsT")
    xT = [persist.tile([FROWS, N], BF16, name=f"xT{b}", tag=f"xT{b}")
          for b in range(BL)]
    # the PE-unblocking loads issue on three different queues so their
    # HWDGE descriptor slots pack back-to-back from t=0; the ACT queue has
    # no preamble, so the first QK tile's feat rows land there first
    wpack = const.tile([FD + SD, 961], BF16, name="c_wpack")
    nc.scalar.dma_start(out=fsT[0:FD, 0:NTW], in_=d["fsT"][0:FD, 0:NTW])
    nc.sync.dma_start(out=wpack, in_=d["wpack"])
    nc.scalar.dma_start(out=fsT[0:FD, NTW:], in_=d["fsT"][0:FD, NTW:])
    nc.gpsimd.dma_start(out=xT[0], in_=d["xT"][0])
    bpack = cload("bpack")
    ones8 = cload("ones8")
    wq = wpack[0:FD, 0:64]
    wk = wpack[0:FD, 64:128]
    ws1 = wpack[:, 128:192]
    kkall = wpack[0:FROWS, 192:576]
    kk0 = wpack[0:FROWS, 576:704]
    kcall = wpack[0:FROWS, 704:896]
    kc0 = wpack[0:FROWS, 896:960]
    ws2 = wpack[0:U, 960:961]
    bruh = bpack[:, 0:1]
    bc2 = bpack[:, 1:2]
    bs1v = bpack[0:U, 2:3]
    bs2v = bpack[0:1, 3:4]
    nc.sync.dma_start(out=xT[1], in_=d["xT"][1])
    nc.sync.dma_start(out=fsT[FD:, :], in_=d["fsT"][FD:, :])
    _adj_cache = {}

    def adjslice(name, t):
        # persist: each slice is DMA'd once (phase 1) and reused in phase 2
        if (name, t) not in _adj_cache:
            sl = d[name][:, t * NTW:(t + 1) * NTW]
            a = persist.tile([NBW, NB, NTW], FP8, name=f"{name}_{t}",
                             tag=f"{name}_{t}")
            nc.sync.dma_start(out=a,
                              in_=sl.rearrange("(j p) w -> p j w", p=NBW))
            _adj_cache[(name, t)] = a
        return _adj_cache[(name, t)]

    adjslice("a1T", 0)
    nc.sync.dma_start(out=xT[2], in_=d["xT"][2])
    adjslice("a2T", 0)
    nc.sync.dma_start(out=xT[3], in_=d["xT"][3])
    # sentinel row: folded into the colsum as a rank-1 matmul against a
    # single ones8 row, so no zero-filled [128, N] tile (or its memset)
    s8row = const.tile([1, N], FP8, name="s8row")
    one_bc = const.tile([128, NTW], F32, name="one_bc")
    nc.vector.memset(one_bc, 1.0)
    # preload the exp_and_others ACT table at t~0 (a real activation would
    # otherwise pay the 1.3us table load right when QT evacuations start)
    actwarm = const.tile([1, 1], F32, name="actwarm")
    nc.scalar.activation(actwarm, one_bc[0:1, 0:1], AF.Exp)

    QT = persist.tile([U // 2, 2, N], FP8, name="QT", tag="QT")
    KT = persist.tile([U // 2, 2, N], FP8, name="KT", tag="KT")
    # resident raw attention support e^T = exp(KQ^T/8), fp8
    et = [persist.tile([NBW, NB, NTW], FP8, name=f"et{t}", tag=f"et{t}")
          for t in range(NT)]
    # rdbc[t][p, n] = 64/d[n]: e-term normalizer, applied to PSUM e-partials
    rdbc = [persist.tile([NBW, NTW], F32, name=f"rdbc{t}", tag=f"rdbc{t}")
            for t in range(NT)]

    # ---- prelude thunks: Q^T, K^T (critical: gate eg0) and the sentinel
    # s chain (only needed by the end of each colsum). QK evacuations split
    # DVE/ACT; the sentinel chain evacuates on the otherwise-idle Pool. ----
    def qk_thunks(t):
        sl = slice(t * NTW, (t + 1) * NTW)

        def t_pk():
            pk = pscr.tile([U, NTW], F32, name="pk", tag="scr")
            nc.tensor.matmul(pk, wk, fsT[0:FD, sl], start=True, stop=True)
            nc.vector.tensor_scalar(KT[:, 0, sl], pk[0:U // 2, :], 8.0, 0.0,
                                    mybir.AluOpType.mult, mybir.AluOpType.max)
            nc.scalar.activation(KT[:, 1, sl], pk[U // 2:U, :], AF.Relu,
                                 scale=8.0)

        def t_pq():
            pq = pscr.tile([U, NTW], F32, name="pq", tag="scr")
            nc.tensor.matmul(pq, wq, fsT[0:FD, sl], start=True, stop=True)
            nc.vector.tensor_scalar(QT[:, 0, sl], pq[0:U // 2, :], 8.0, 0.0,
                                    mybir.AluOpType.mult, mybir.AluOpType.max)
            nc.scalar.activation(QT[:, 1, sl], pq[U // 2:U, :], AF.Relu,
                                 scale=8.0)

        return [t_pk, t_pq]

    def s_thunks(t):
        sl = slice(t * NTW, (t + 1) * NTW)
        s1t = stage.tile([U, NTW], BF16, name="s1t", tag="sig")

        def t_ps1():
            ps1 = pscr.tile([U, NTW], F32, name="ps1", tag="scr")
            nc.tensor.matmul(ps1, ws1, fsT[:, sl], start=True, stop=True)
            if t == 0:
                nc.vector.tensor_scalar(s1t, ps1, bs1v, 0.0,
                                        mybir.AluOpType.add,
                                        mybir.AluOpType.max)
            else:
                nc.scalar.activation(s1t, ps1, AF.Relu, bias=bs1v)

        def t_ps2():
            ps2 = pscr.tile([1, NTW], F32, name="ps2", tag="scr")
            nc.tensor.matmul(ps2, ws2, s1t, start=True, stop=True)
            if t == 0:
                nc.vector.tensor_scalar(s8row[0:1, sl], ps2, bs2v, 0.0,
                                        mybir.AluOpType.add,
                                        mybir.AluOpType.max)
            else:
                nc.scalar.activation(s8row[0:1, sl], ps2, AF.Relu,
                                     bias=bs2v)

        return [t_ps1, t_ps2]

    # ---- tile-0 e-generation: singles on one rotating bank, each QK
    # matmul evacuated by its own exp. The 16 serial exps are tile-0's
    # critical path; the PE stream runs y-generation between them. ----
    def eg0_thunks():
        sl = slice(0, NTW)
        eg0 = pse.tile([NBW, 2, NTW], F32, name="eg0", tag="pse")

        def mk(j):
            def f():
                pej = eg0[:, j % 2, :]
                nc.tensor.matmul(pej, KT[:, :, j * NBW:(j + 1) * NBW],
                                 QT[:, :, sl], start=True, stop=True,
                                 perf_mode=DR)
                nc.scalar.activation(et[0][:, j, :], pej, AF.Exp,
                                     scale=0.125 / 64.0)
            return f
        return [mk(j) for j in range(NB)]

    # ---- e-generation for tiles 1..3: pairs with one batched exp ----
    def eg_pair_thunks(t):
        sl = slice(t * NTW, (t + 1) * NTW)

        def mk(j):
            def f():
                pp = pse.tile([NBW, 2, NTW], F32, name="pp", tag="pse")
                nc.tensor.matmul(pp[:, 0, :], KT[:, :, j * NBW:(j + 1) * NBW],
                                 QT[:, :, sl], start=True, stop=True,
                                 perf_mode=DR)
                nc.tensor.matmul(pp[:, 1, :],
                                 KT[:, :, (j + 1) * NBW:(j + 2) * NBW],
                                 QT[:, :, sl], start=True, stop=True,
                                 perf_mode=DR)
                nc.scalar.activation(et[t][:, j:j + 2, :], pp, AF.Exp,
                                     scale=0.125 / 64.0)
            return f
        return [mk(j) for j in range(0, NB, 2)]

    def colsum_thunks(t):
        """rdbc[t][p, n] = 64 / (s[n] + colsum(e^T)[n]), all partitions."""
        sl = slice(t * NTW, (t + 1) * NTW)
        pcs = pscr.tile([NBW, NTW], F32, name="pcs", tag="scr")
        th = []
        for k in range(NP):
            def f(k=k):
                nc.tensor.matmul(pcs, ones8, et[t][:, 2 * k:2 * k + 2, :],
                                 start=(k == 0), stop=False, perf_mode=DR)
            th.append(f)
        th.append(lambda: nc.tensor.matmul(
            pcs, ones8[0:1, 0, :], s8row[0:1, sl], start=False, stop=True))
        th.append(lambda: nc.vector.reciprocal(rdbc[t], pcs))
        return th

    # ---- phase-1 Y tiles: Y[m,b] = X_b @ [8*k_r[m]|8*k_u[m]] ----
    # stored fp8 [node%128, node//128, m, u']; copies split DVE/ACT
    y = [ypool.tile([NBW, NB, 3, 2 * U], FP8, name=f"y_{b}", tag=f"y{b}")
         for b in range(BL)]

    # drain-engine assignment per batch (even-j, odd-j): chosen so each
    # prologue engine's queue stays under the ~17us exp-bound window, and
    # ACT only gets drains whose emission lands after the exp stream.
    Y_ENG = {0: ("dve", "dve"), 1: ("dve", "act"),
             2: ("dve", "dve"), 3: ("dve", "act")}

    def ygen_thunks(b):
        # batches 0-2 run in the prologue, where the psacc banks are still
        # idle: alternating psacc/pscr gives a 6-deep scratch rotation so
        # lagging drains never head-block the PE stream.
        def mk(j):
            def f():
                nsl = slice(j * NBW, (j + 1) * NBW)
                if b < 3 and j % 2 == 1:
                    py = psacc.tile([NBW, 3 * 2 * U], F32, name="py",
                                    tag="acc")
                else:
                    py = pscr.tile([NBW, 3 * 2 * U], F32, name="py",
                                   tag="scr")
                nc.tensor.matmul(py, xT[b][:, nsl], kkall, start=True,
                                 stop=True)
                eng = Y_ENG[b][j % 2]
                src = py.rearrange("p (m u) -> p m u", m=3)
                if eng == "dve":
                    nc.vector.tensor_copy(y[b][:, j, :, :], src)
                else:
                    nc.scalar.activation(
                        y[b][:, j, :, :].rearrange("p m u -> p (m u)"),
                        py, AF.Copy)
            return f
        return [mk(j) for j in range(NB)]

    def interleave(main, extra, ratio=2):
        mi = ei = 0
        while mi < len(main) or ei < len(extra):
            for _ in range(ratio):
                if mi < len(main):
                    main[mi](); mi += 1
            if ei < len(extra):
                extra[ei](); ei += 1

    # =================== phase 1: r & u gates ===================
    # Each gate group: e-support contraction first (own PSUM group), then a
    # DVE multiply by rdbc[t] on the PSUM partial, then identity + adjacency
    # terms accumulate on top (start=False).
    def e_thunks(yt, t, pa):
        th = []
        for k in range(NP):
            def f(k=k):
                nc.tensor.matmul(pa, yt[:, 2 * k:2 * k + 2, 2, :],
                                 et[t][:, 2 * k:2 * k + 2, :],
                                 start=(k == 0), stop=(k == NP - 1),
                                 perf_mode=DR)
            th.append(f)
        return th

    def rest_thunks1(b, t, sl, a1, a2, pa):
        th = [lambda: nc.vector.tensor_mul(pa, pa, rdbc[t]),
              lambda: nc.tensor.matmul(pa, kk0, xT[b][:, sl], start=False,
                                       stop=False, skip_group_check=True)]
        for m, mov in ((0, a1), (1, a2)):
            for k in range(NP):
                def f(m=m, mov=mov, k=k):
                    nc.tensor.matmul(pa, y[b][:, 2 * k:2 * k + 2, m, :],
                                     mov[:, 2 * k:2 * k + 2, :],
                                     start=False,
                                     stop=(m == 1 and k == NP - 1),
                                     perf_mode=DR, skip_group_check=True)
                th.append(f)
        return th

    def gate1(b, t, sl, pa):
        def f():
            # sigmoid(z) computed as 0.5 + 0.5*tanh(z/2): keeps every ACT
            # function in the exp_and_others table (no table reloads).
            # sig holds T = tanh(z/2) = 2*sigmoid(z) - 1.
            sig = stage.tile([128, NTW], F32, name="sig", tag="sig")
            nc.scalar.activation(sig, pa, AF.Tanh, scale=GATE_SCALE / 2,
                                 bias=bruh)
            # rows 0:64 in place: (T_r + 1)*h = 2*r*h; the c-pass kernels'
            # h-rows carry a compensating 1/2. u kept as raw T_u.
            rtmp = stage.tile([U, NTW], F32, name="rtmp", tag="rtmp")
            nc.gpsimd.tensor_add(rtmp, sig[0:U, :], one_bc[0:U, :])
            nc.gpsimd.tensor_mul(xT[b][0:U, sl], rtmp, xT[b][0:U, sl])
            dst = usig[t][b // 2][(b % 2) * U:(b % 2 + 1) * U, :]
            if b % 2 == 0:
                # cross-partition move (rows 64:128 -> 0:64): DVE only
                nc.vector.tensor_copy(dst, sig[U:128, :])
            else:
                nc.gpsimd.tensor_copy(dst, sig[U:128, :])
        return [f]

    # u stays resident in SBUF (T_u rows per (tile, batch-pair)): no
    # uscr DRAM round-trip, no phase-2 up prefetch DMAs
    usig = [[persist.tile([128, NTW], F32, name=f"usig{t}_{p}",
                          tag=f"us{t}{p}") for p in range(2)]
            for t in range(NT)]

    yc = [None, None]

    def ycgen_half(yct, p, half):
        b = 2 * p + half
        usl = slice(half * U, (half + 1) * U)
        th = []
        for j in range(NB):
            def f(b=b, usl=usl, j=j, yct=yct, half=half):
                nsl = slice(j * NBW, (j + 1) * NBW)
                pyc = pscr.tile([NBW, 3 * U], F32, name="pyc", tag="scr")
                nc.tensor.matmul(pyc, xT[b][:, nsl], kcall,
                                 start=True, stop=True)
                dst = yct[:, j, :, usl]
                src = pyc.rearrange("p (m u) -> p m u", m=3)
                if (half * NB + j) % 2 == 0:
                    nc.vector.tensor_copy(dst, src)
                else:
                    nc.scalar.activation(dst, src, AF.Copy)
            th.append(f)
        return th

    def ycgen_thunks(yct, p):
        return ycgen_half(yct, p, 0) + ycgen_half(yct, p, 1)

    # ---------- prologue ----------
    # Each QK-prelude tile is chased immediately by its four tile-0 QK
    # blocks (subtile deps let eg0[j] start as soon as its K block lands),
    # with the colsum chasing each exp pair and y-generation as PE filler.
    # The serial exp stream on ACT is tile-0's critical path; everything
    # else here is arranged to keep it back-to-back.
    qk = []
    for t in range(NT):
        qk += qk_thunks(t)
    interleave(qk, ygen_thunks(0), ratio=1)
    eg0 = eg0_thunks()
    cs0 = colsum_thunks(0)
    crit = []
    for k in range(NP):
        crit += [eg0[2 * k], eg0[2 * k + 1], cs0[k]]
    crit += cs0[NP:]
    sth = []
    for t in range(NT):
        sth += s_thunks(t)
    filler = []
    yg1 = ygen_thunks(1)
    for i, f in enumerate(yg1):
        filler.append(f)
        if i % 2 == 1 and sth:
            filler.append(sth.pop(0))
    filler += sth
    interleave(filler, crit, ratio=2)
    # tile-1's QK pairs ride the late prologue (their pse slots free as
    # tile-0's exps complete; their exps follow et0's on ACT) so rdbc[1]
    # is ready well before tile 1 needs it
    interleave(ygen_thunks(2), eg_pair_thunks(1), ratio=2)

    for t in range(NT):
        sl = slice(t * NTW, (t + 1) * NTW)
        a1 = adjslice("a1T", t)
        a2 = adjslice("a2T", t)
        pa = [psacc.tile([128, NTW], F32, name="pa", tag="acc")
              for _ in range(BL)]
        # schedule: e-streams lead their group's rest by one slot so the
        # DVE rdbc-multiply hides under the next group's e-stream.
        main = []
        main += e_thunks(y[0], t, pa[0]) + e_thunks(y[1], t, pa[1])
        if t == 0:
            # batch-3 y-generation woven into the rdbc-gated stretch
            m1 = e_thunks(y[2], t, pa[2]) \
                + rest_thunks1(0, t, sl, a1, a2, pa[0]) \
                + gate1(0, t, sl, pa[0])
            mi = []
            yg3 = ygen_thunks(3)
            k = 0
            for i, f in enumerate(m1):
                mi.append(f)
                while k < len(yg3) * (i + 1) // len(m1):
                    mi.append(yg3[k]); k += 1
            mi.extend(yg3[k:])
            main += mi
        else:
            main += e_thunks(y[2], t, pa[2])
            main += rest_thunks1(0, t, sl, a1, a2, pa[0]) \
                + gate1(0, t, sl, pa[0])
        m2 = e_thunks(y[3], t, pa[3]) \
            + rest_thunks1(1, t, sl, a1, a2, pa[1]) + gate1(1, t, sl, pa[1])
        post = []
        post += rest_thunks1(2, t, sl, a1, a2, pa[2]) + gate1(2, t, sl, pa[2])
        post += rest_thunks1(3, t, sl, a1, a2, pa[3]) + gate1(3, t, sl, pa[3])
        extra = eg_pair_thunks(t + 1) if 0 < t < NT - 1 else []
        if t == NT - 1:
            # yc[0] reads post-rh xT rows: batch 0's half can weave as soon
            # as gate1(0) has run, batch 1's into the post stream.
            yc[0] = ypool.tile([NBW, NB, 3, 2 * U], FP8, name="yc_0",
                               tag="y0")
            interleave(main, [])
            interleave(m2, ycgen_half(yc[0], 0, 0), ratio=1)
            interleave(post, ycgen_half(yc[0], 0, 1), ratio=1)
        else:
            allm = main + m2 + post
            alle = extra
            interleave(allm, alle, ratio=max(1, len(allm) // max(1, len(alle))))
        # colsum for t+1 at tile end: its exp deps had the whole tile to run
        if t + 1 < NT:
            for f in colsum_thunks(t + 1):
                f()

    # =================== phase 2+3: c gate & h_new ===================
    yc[1] = ypool.tile([NBW, NB, 3, 2 * U], FP8, name="yc_1", tag="y1")
    a1 = adjslice("a1T", 0)
    a2 = adjslice("a2T", 0)

    def hu_prefetch(p, t, sl, hu):
        # single 2-batch DMA each for h and u: [2, 64, w] -> [128, w]
        b0 = 2 * p

        def f():
            hp = p3p.tile([128, NTW], F32, name="hp", tag="hp")
            nc.sync.dma_start(
                out=hp, in_=d["hT"][b0:b0 + 2, :, sl].rearrange(
                    "b u w -> (b u) w"))
            hu.extend((hp, usig[t][p]))
        return [f]

    def rest_thunks2(p, t, sl, a1, a2, pa, hu):
        b0, b1 = 2 * p, 2 * p + 1

        th = [lambda: nc.vector.tensor_mul(pa, pa, rdbc[t]),
              lambda: nc.tensor.matmul(pa[0:U, :], kc0, xT[b0][:, sl],
                                       start=False, stop=False,
                                       skip_group_check=True),
              lambda: nc.tensor.matmul(pa[U:128, :], kc0, xT[b1][:, sl],
                                       start=False, stop=False,
                                       skip_group_check=True)]
        for m, mov in ((0, a1), (1, a2)):
            for k in range(NP):
                def f(m=m, mov=mov, k=k):
                    nc.tensor.matmul(pa, yc[p][:, 2 * k:2 * k + 2, m, :],
                                     mov[:, 2 * k:2 * k + 2, :],
                                     start=False,
                                     stop=(m == 1 and k == NP - 1),
                                     perf_mode=DR, skip_group_check=True)
                th.append(f)
        return th

    def tail2(p, t, sl, pa, hu, splits=2):
        b0, b1 = 2 * p, 2 * p + 1

        def f():
            hp, up = hu
            ct = stage.tile([128, NTW], F32, name="ct", tag="sig")
            t1 = p3p.tile([128, NTW], F32, name="t1", tag="t1")
            # compute in column chunks so ACT/Pool/DVE stages pipeline, but
            # write out full-width (HWDGE descriptor slots are the scarce
            # resource at the drain)
            for c0 in range(0, NTW, NTW // splits):
                cs = slice(c0, c0 + NTW // splits)
                nc.scalar.activation(ct[:, cs], pa[:, cs], AF.Tanh,
                                     scale=GATE_SCALE, bias=bc2)
                # h_new = c + u*(h-c) with up = T_u = 2u-1:
                # t1 = h-c (Pool); t1 = (T_u+1)*t1; t1 = t1/2 + c (DVE)
                nc.gpsimd.tensor_sub(t1[:, cs], hp[:, cs], ct[:, cs])
                nc.vector.scalar_tensor_tensor(
                    t1[:, cs], up[:, cs], 1.0, t1[:, cs],
                    mybir.AluOpType.add, mybir.AluOpType.mult)
                nc.vector.scalar_tensor_tensor(
                    t1[:, cs], t1[:, cs], 0.5, ct[:, cs],
                    mybir.AluOpType.mult, mybir.AluOpType.add)
                if splits == 4 and c0 + NTW // splits in (NTW // 2, NTW):
                    hs = slice(c0 + NTW // splits - NTW // 2,
                               c0 + NTW // splits)
                    # the two final half-writes issue on different queues
                    # so their HWDGE/issue pipelines overlap
                    eng = nc.sync if hs.start == 0 else nc.scalar
                    eng.dma_start(
                        out=out_h[b0:b0 + 2, :,
                                  t * NTW + hs.start:t * NTW + hs.stop]
                        .rearrange("b u w -> (b u) w"),
                        in_=t1[:, hs])
            if splits != 4:
                nc.sync.dma_start(
                    out=out_h[b0:b0 + 2, :, sl].rearrange("b u w -> (b u) w"),
                    in_=t1)
        return [f]

    prev = []
    pa_pre = pa_pre1 = None
    for t in range(NT):
        sl = slice(t * NTW, (t + 1) * NTW)
        if t > 0:
            a1 = adjslice("a1T", t)
            a2 = adjslice("a2T", t)
        pa0 = pa_pre if pa_pre is not None else \
            psacc.tile([128, NTW], F32, name="pa2", tag="acc")
        pa1 = pa_pre1 if pa_pre1 is not None else \
            psacc.tile([128, NTW], F32, name="pa2", tag="acc")
        hu0, hu1 = [], []
        main = hu_prefetch(0, t, sl, hu0) + prev + hu_prefetch(1, t, sl, hu1)
        if t == 0:
            # p0's whole contraction only needs yc[0] (built at the end of
            # phase 1): run it as the dense stream with yc[1] generation
            # woven in as drain-paced filler, then chase e(yc[1]).
            p0w = e_thunks(yc[0], t, pa0) \
                + rest_thunks2(0, t, sl, a1, a2, pa0, hu0)
            ycg = ycgen_thunks(yc[1], 1)
            mi = []
            k = 0
            for i, f in enumerate(p0w):
                mi.append(f)
                while k < len(ycg) * (i + 1) // len(p0w):
                    mi.append(ycg[k]); k += 1
            main += mi + ycg[k:] + e_thunks(yc[1], t, pa1)
        else:
            if pa_pre is None:
                main += e_thunks(yc[0], t, pa0)
            if pa_pre1 is None:
                main += e_thunks(yc[1], t, pa1)
            main += rest_thunks2(0, t, sl, a1, a2, pa0, hu0)
        # pre-stream the next tile's e-contractions: PE filler while this
        # tile's tanh/tail chains drain (p1's bank alternates between the
        # phase-2-idle pse pool and a dedicated pscr tag)
        if t + 1 < NT:
            pa_pre = psacc.tile([128, NTW], F32, name="pa2", tag="acc")
            main += e_thunks(yc[0], t + 1, pa_pre)
            if t % 2 == 0:
                pa_pre1 = pse.tile([128, NTW], F32, name="pa2e", tag="pse")
            else:
                pa_pre1 = psacc.tile([128, NTW], F32, name="pa2e", tag="acc")
            main += e_thunks(yc[1], t + 1, pa_pre1)
        else:
            pa_pre = pa_pre1 = None
        main += tail2(0, t, sl, pa0, hu0)
        for f in main:
            f()
        # carry p=1's rest into the next tile so its DVE multiply and the
        # tail chain overlap with the next tile's e-streams
        prev = rest_thunks2(1, t, sl, a1, a2, pa1, hu1) \
            + tail2(1, t, sl, pa1, hu1, splits=4 if t == NT - 1 else 2)
    for f in prev:
        f()

    ctx.close()


_CACHE = {}


def _get_program():
    if "nc" not in _CACHE:
        _CACHE["nc"] = _build_program()
    return _CACHE["nc"]


def _prep_inputs(inputs, h_prev, adj1, adj2, feat, SE, Wq, Wk, Ws1, bs1, Ws2,
                 bs2, r_kernel, r_bias, u_kernel, u_bias, c_kernel, c_bias):
    bf = ml_dtypes.bfloat16
    f8 = ml_dtypes.float8_e4m3fn
    f32 = np.float32
    perm = list(range(DIN, FROWS)) + list(range(DIN))  # [h(64); inputs(2)]

    h3 = np.asarray(h_prev, f32).reshape(B, N, U)
    hT = np.ascontiguousarray(h3.transpose(0, 2, 1))            # [B, U, N]
    inT = np.asarray(inputs, f32).transpose(0, 2, 1)            # [B, DIN, N]
    xT = np.concatenate([hT, inT], axis=1).astype(bf)           # [B, 66, N]

    rk = np.asarray(r_kernel, f32)[:, perm, :]
    uk = np.asarray(u_kernel, f32)[:, perm, :]
    ck = np.asarray(c_kernel, f32)[:, perm, :]
    kkall = np.concatenate(
        [np.concatenate([rk[m], uk[m]], axis=1) for m in (1, 2, 3)],
        axis=1) * SC_Y                                          # [66, 384]
    kk0 = np.concatenate([rk[0], uk[0]], axis=1) * SC_ID        # [66, 128]
    # c-pass x_cat holds 2*r*h in its h-rows (r comes from the tanh-form
    # sigmoid as T_r + 1); compensate with a 1/2 on the kernels' h-rows.
    ck = ck.copy()
    ck[:, 0:U, :] *= 0.5
    kcall = np.concatenate([ck[1], ck[2], ck[3]], axis=1) * SC_Y
    kc0 = ck[0] * SC_ID

    wpk = np.zeros((FD + SD, 961), f32)
    wpk[0:FD, 0:64] = np.asarray(Wq, f32)
    wpk[0:FD, 64:128] = np.asarray(Wk, f32)
    wpk[:, 128:192] = np.asarray(Ws1, f32)
    wpk[0:FROWS, 192:576] = kkall
    wpk[0:FROWS, 576:704] = kk0
    wpk[0:FROWS, 704:896] = kcall
    wpk[0:FROWS, 896:960] = kc0
    wpk[0:U, 960] = np.asarray(Ws2, f32).reshape(U)
    bpk = np.zeros((2 * U, 4), f32)
    bpk[:, 0] = np.concatenate([np.asarray(r_bias, f32).mean(0),
                                np.asarray(u_bias, f32).mean(0)]) * 0.5
    bpk[:, 1] = np.tile(np.asarray(c_bias, f32).mean(0), 2)
    bpk[0:U, 2] = np.asarray(bs1, f32)
    bpk[0, 3] = np.asarray(bs2, f32).reshape(1)[0]

    shared = {
        "a1T": np.ascontiguousarray(np.asarray(adj1, f32).T * SC_ADJ).astype(f8),
        "a2T": np.ascontiguousarray(np.asarray(adj2, f32).T * SC_ADJ).astype(f8),
        "fsT": np.ascontiguousarray(
            np.concatenate([np.asarray(feat, f32).T, np.asarray(SE, f32).T],
                           axis=0)).astype(bf),
        "wpack": wpk.astype(bf),
        "bpack": bpk,
        "ones8": np.full((NBW, 2, NBW), 1.0 / 64.0, f8),
    }
    in_maps = []
    for c in range(NCORES):
        bsl = slice(c * BL, (c + 1) * BL)
        m = dict(shared)
        m["xT"] = np.ascontiguousarray(xT[bsl])
        m["hT"] = np.ascontiguousarray(hT[bsl])
        in_maps.append(m)
    return in_maps


def kernel(**inputs):
    os.environ.setdefault("NEURON_RT_RESET_CORES", "1")
    nc = _get_program()
    in_maps = _prep_inputs(**inputs)
    res = None
    err = None
    for _ in range(2):
        try:
            res = run_bass_kernel_spmd(nc, in_maps, list(range(NCORES)))
            break
        except Exception as e:  # e.g. a wedged device; retry once
            err = e
    if res is None:
        raise err
    outs = []
    for c in range(NCORES):
        o = res.results[c]["out"]                     # [BL, U, N] f32
        outs.append(o.transpose(0, 2, 1).reshape(BL, N * U))
    return np.concatenate(outs, axis=0).astype(np.float32)


# revision 120
# speedup vs baseline: 1.0037x; 1.0012x over previous
"""MFGCGRU (graph-conv GRU cell) Trainium2 kernel.

Strategy: data-parallel over batch B=32 across 8 NeuronCores (4 batches
per core), NxN supports replicated. The diffusion conv is kernel-first:
S_m @ (X @ k_m), with the node contractions run as fp8e4m3 DoubleRow
matmuls (2 K-blocks per instruction at 0.5 cycles/row = 4x bf16 MAC
throughput). fp8's narrow exponent range is handled by host-side
power-of-two scaling:

  - adjacency S^T stored fp8 at x64,
  - Y = X @ (8 x kernel) quantized to fp8 (so adj-terms come out x512),
  - identity-path kernels kk0/kc0 stored bf16 at x512,
  - the attention support stays raw in fp8 (e = exp(64*QK/8) written by
    ACT straight off the QK PSUM, itself an fp8 DoubleRow matmul over
    u-halves). Its normalizer rdbc = 64/(s + rowsum(e)) is produced as a
    full [128, n] broadcast by a DoubleRow colsum against a constant 1/64
    tile, with the learned sentinel s folded in as a rank-1 matmul of a
    single fp8 row against one ones8 row; each gate group contracts e
    FIRST into its PSUM bank, multiplies the partial by rdbc on DVE, then
    accumulates identity + adjacency terms on top (x8 y-scale x64
    adj-scale = x512 everywhere).
  - gates read PSUM directly: both sigmoids are evaluated as
    0.5 + 0.5*tanh(z/2) so Relu/Exp/Tanh/Copy all live in one ACT
    function table (one LoadActFuncSet, pre-warmed at t~0 by a dummy
    exp); the 0.5s fold into the c-kernels' h-rows and the GRU tail's
    fused scalar_tensor_tensor ops.

The attention prelude (Q/K/sentinel) runs in bf16 (fp32 matmuls cost 4
cycles/row on PE vs 1 for bf16; Q/K are quantized to fp8 anyway so bf16
costs nothing numerically). All small weights ship in one packed DMA
(wpack/bpack) because each dma_start costs ~650ns of sequencer issue +
~630ns of HWDGE time; the three front loads issue on the SP, ACT and
Pool(SWDGE) queues in parallel. The prologue is built around tile-0's
serial exp chain on ACT: the eight QK matmuls run back-to-back first,
the 16 tile-0 QK-block matmuls ping-pong the two pse banks with the
colsum chasing each exp pair, and batch 0-2 Y-generation fills the PE
stream (its PSUM scratch rotates over psacc+pscr, six banks deep, so
lagging drains never head-block the in-order stream). Batch-3 Y and the
tile-0 e-contractions chase the exp stream into tile 0. Later tiles
batch their exps in pairs and pre-stream the next tile's QK + colsum.
At the phase boundary, yc[0] generation weaves in right behind the last
tile's r-updates, and phase-2 tile 0 runs pair-0's whole contraction
(only needs yc[0]) as the dense stream while yc[1] generates behind it.

e (4.2MB) and both adjacency operands (8.4MB) stay resident in SBUF so
exp and the adjacency DMAs run once across both passes. PSUM->SBUF
evacuations live on DVE and ACT only (GPSIMD cannot touch PSUM on real
hardware, whatever the cost model thinks); the SBUF-only elementwise
work (r*h, GRU tail subtract) runs on the otherwise idle Pool engine.
"""

import contextlib
import os

import numpy as np
import ml_dtypes

import concourse.bass as bass
import concourse.bacc as bacc
import concourse.tile as tile
from concourse import mybir
from concourse.bass_utils import run_bass_kernel_spmd

F32 = mybir.dt.float32
BF16 = mybir.dt.bfloat16
FP8 = mybir.dt.float8e4
AF = mybir.ActivationFunctionType
DR = mybir.MatmulPerfMode.DoubleRow

B, N, DIN, U, FD, SD = 32, 2048, 2, 64, 32, 64
NCORES = 8
BL = B // NCORES          # batches per core
NTW = 512                 # n-tile width
NT = N // NTW             # 4 n-tiles
NBW = 128                 # node-block width
NB = N // NBW             # 16 node blocks
NP = NB // 2              # 8 node-block pairs (DoubleRow)
FROWS = DIN + U           # 66

SC_ADJ = 64.0             # host scale on adjacency (fp8)
SC_Y = 8.0                # host scale on y kernels (fp8 y tiles)
SC_ID = 512.0             # host scale on identity kernels (bf16)
GATE_SCALE = 0.25 / 512.0 # sigmoid/tanh pre-scale: mean over 4 supports / 512


def _build_program():
    nc = bacc.Bacc("TRN2", debug=False, num_devices=NCORES)

    d = {}

    def din(name, shape, dt):
        d[name] = nc.dram_tensor(name, shape, dt, kind="ExternalInput").ap()

    din("xT", [BL, FROWS, N], BF16)
    din("hT", [BL, U, N], F32)
    din("a1T", [N, N], FP8)
    din("a2T", [N, N], FP8)
    din("fsT", [FD + SD, N], BF16)
    # all small bf16 weights packed column-wise into one DMA:
    # wq 0:64 | wk 64:128 | ws1 128:192 | kkall 192:576 | kk0 576:704 |
    # kcall 704:896 | kc0 896:960 | ws2 960:961
    din("wpack", [FD + SD, 961], BF16)
    # f32 bias columns: bruh | bc2 | bs1v | bs2v
    din("bpack", [2 * U, 4], F32)
    din("ones8", [NBW, 2, NBW], FP8)        # constant 1/64
    out_h = nc.dram_tensor("out", [BL, U, N], F32, kind="ExternalOutput").ap()
    uscr = nc.dram_tensor("uscr", [BL, U, N], F32).ap()

    with tile.TileContext(nc) as tc, \
            nc.allow_low_precision(reason="fp8 support contraction by design"):
        _emit(tc, d, out_h, uscr)
    nc.compile()
    return nc


def _emit(tc, d, out_h, uscr):
    nc = tc.nc
    ctx = contextlib.ExitStack()
    const = ctx.enter_context(tc.tile_pool(name="const", bufs=1))
    persist = ctx.enter_context(tc.tile_pool(name="persist", bufs=1))
    ypool = ctx.enter_context(tc.tile_pool(name="ypool", bufs=1))
    stage = ctx.enter_context(tc.tile_pool(name="stage", bufs=2))
    p3p = ctx.enter_context(tc.tile_pool(name="p3p", bufs=3))
    # PSUM budget is 8 banks (16KB/partition):
    #   psacc: 3 x 1 bank  (gate accumulators)
    #   pse:   1 x 2 banks (QK-pair scratch in phase 1, spare acc in phase 2)
    #   pscr:  "scr" 3 x 1 bank (y/yc/prelude/colsum scratch)
    psacc = ctx.enter_context(tc.tile_pool(name="psacc", bufs=3, space="PSUM"))
    pse = ctx.enter_context(tc.tile_pool(name="pse", bufs=1, space="PSUM"))
    pscr = ctx.enter_context(tc.tile_pool(name="pscr", bufs=3, space="PSUM"))

    # ---- constants / weights in SBUF ----
    def cload(name):
        ap = d[name]
        t = const.tile(list(ap.shape), ap.dtype, name=f"c_{name}")
        nc.sync.dma_start(out=t, in_=ap)
        return t

    # DMA order = earliest-consumer order, with the two PE-unblocking
    # tensors (fsT's feat rows for the QK prelude, xT0 for y-generation)
    # chunked so the first matmuls start early. Small weights ride in one
    # packed DMA (each separate DMA costs ~700ns of queue time).
    fsT = const.tile([FD + SD, N], BF16, name="c_fsT")
    xT = [persist.tile([FROWS, N], BF16, name=f"xT{b}", tag=f"xT{b}")
          for b in range(BL)]
    # the PE-unblocking loads issue on three different queues so their
    # HWDGE descriptor slots pack back-to-back from t=0; the ACT queue has
    # no preamble, so the first QK tile's feat rows land there first
    wpack = const.tile([FD + SD, 961], BF16, name="c_wpack")
    nc.scalar.dma_start(out=fsT[0:FD, 0:NTW], in_=d["fsT"][0:FD, 0:NTW])
    nc.sync.dma_start(out=wpack, in_=d["wpack"])
    nc.scalar.dma_start(out=fsT[0:FD, NTW:], in_=d["fsT"][0:FD, NTW:])
    nc.gpsimd.dma_start(out=xT[0], in_=d["xT"][0])
    bpack = cload("bpack")
    ones8 = cload("ones8")
    wq = wpack[0:FD, 0:64]
    wk = wpack[0:FD, 64:128]
    ws1 = wpack[:, 128:192]
    kkall = wpack[0:FROWS, 192:576]
    kk0 = wpack[0:FROWS, 576:704]
    kcall = wpack[0:FROWS, 704:896]
    kc0 = wpack[0:FROWS, 896:960]
    ws2 = wpack[0:U, 960:961]
    bruh = bpack[:, 0:1]
    bc2 = bpack[:, 1:2]
    bs1v = bpack[0:U, 2:3]
    bs2v = bpack[0:1, 3:4]
    nc.sync.dma_start(out=xT[1], in_=d["xT"][1])
    nc.sync.dma_start(out=fsT[FD:, :], in_=d["fsT"][FD:, :])
    _adj_cache = {}

    def adjslice(name, t):
        # persist: slices are DMA'd once as tile-PAIRS (halving the DMA
        # issue/HWDGE count) and reused in phase 2; callers get an AP view
        pr = t // 2
        if (name, pr) not in _adj_cache:
            sl = d[name][:, pr * 2 * NTW:(pr + 1) * 2 * NTW]
            a = persist.tile([NBW, NB, 2 * NTW], FP8, name=f"{name}_{pr}",
                             tag=f"{name}_{pr}")
            nc.sync.dma_start(out=a,
                              in_=sl.rearrange("(j p) w -> p j w", p=NBW))
            _adj_cache[(name, pr)] = a
        a = _adj_cache[(name, pr)]
        return a[:, :, (t % 2) * NTW:(t % 2 + 1) * NTW]

    adjslice("a1T", 0)
    nc.sync.dma_start(out=xT[2], in_=d["xT"][2])
    adjslice("a2T", 0)
    nc.sync.dma_start(out=xT[3], in_=d["xT"][3])
    # sentinel row: folded into the colsum as a rank-1 matmul against a
    # single ones8 row, so no zero-filled [128, N] tile (or its memset)
    s8row = const.tile([1, N], FP8, name="s8row")
    one_bc = const.tile([128, NTW], F32, name="one_bc")
    nc.vector.memset(one_bc, 1.0)
    # preload the exp_and_others ACT table at t~0 (a real activation would
    # otherwise pay the 1.3us table load right when QT evacuations start)
    actwarm = const.tile([1, 1], F32, name="actwarm")
    nc.scalar.activation(actwarm, one_bc[0:1, 0:1], AF.Exp)

    QT = persist.tile([U // 2, 2, N], FP8, name="QT", tag="QT")
    KT = persist.tile([U // 2, 2, N], FP8, name="KT", tag="KT")
    # resident raw attention support e^T = exp(KQ^T/8), fp8
    et = [persist.tile([NBW, NB, NTW], FP8, name=f"et{t}", tag=f"et{t}")
          for t in range(NT)]
    # rdbc[t][p, n] = 64/d[n]: e-term normalizer, applied to PSUM e-partials
    rdbc = [persist.tile([NBW, NTW], F32, name=f"rdbc{t}", tag=f"rdbc{t}")
            for t in range(NT)]

    # ---- prelude thunks: Q^T, K^T (critical: gate eg0) and the sentinel
    # s chain (only needed by the end of each colsum). QK evacuations split
    # DVE/ACT; the sentinel chain evacuates on the otherwise-idle Pool. ----
    def qk_thunks(t):
        sl = slice(t * NTW, (t + 1) * NTW)

        def t_pk():
            pk = pscr.tile([U, NTW], F32, name="pk", tag="scr")
            nc.tensor.matmul(pk, wk, fsT[0:FD, sl], start=True, stop=True)
            nc.vector.tensor_scalar(KT[:, 0, sl], pk[0:U // 2, :], 8.0, 0.0,
                                    mybir.AluOpType.mult, mybir.AluOpType.max)
            nc.scalar.activation(KT[:, 1, sl], pk[U // 2:U, :], AF.Relu,
                                 scale=8.0)

        def t_pq():
            pq = pscr.tile([U, NTW], F32, name="pq", tag="scr")
            nc.tensor.matmul(pq, wq, fsT[0:FD, sl], start=True, stop=True)
            nc.vector.tensor_scalar(QT[:, 0, sl], pq[0:U // 2, :], 8.0, 0.0,
                                    mybir.AluOpType.mult, mybir.AluOpType.max)
            nc.scalar.activation(QT[:, 1, sl], pq[U // 2:U, :], AF.Relu,
                                 scale=8.0)

        return [t_pk, t_pq]

    def s_thunks(t):
        sl = slice(t * NTW, (t + 1) * NTW)
        s1t = stage.tile([U, NTW], BF16, name="s1t", tag="sig")

        def t_ps1():
            ps1 = pscr.tile([U, NTW], F32, name="ps1", tag="scr")
            nc.tensor.matmul(ps1, ws1, fsT[:, sl], start=True, stop=True)
            if t == 0:
                nc.vector.tensor_scalar(s1t, ps1, bs1v, 0.0,
                                        mybir.AluOpType.add,
                                        mybir.AluOpType.max)
            else:
                nc.scalar.activation(s1t, ps1, AF.Relu, bias=bs1v)

        def t_ps2():
            ps2 = pscr.tile([1, NTW], F32, name="ps2", tag="scr")
            nc.tensor.matmul(ps2, ws2, s1t, start=True, stop=True)
            if t == 0:
                nc.vector.tensor_scalar(s8row[0:1, sl], ps2, bs2v, 0.0,
                                        mybir.AluOpType.add,
                                        mybir.AluOpType.max)
            else:
                nc.scalar.activation(s8row[0:1, sl], ps2, AF.Relu,
                                     bias=bs2v)

        return [t_ps1, t_ps2]

    # ---- tile-0 e-generation: singles on one rotating bank, each QK
    # matmul evacuated by its own exp. The 16 serial exps are tile-0's
    # critical path; the PE stream runs y-generation between them. ----
    def eg0_thunks():
        sl = slice(0, NTW)
        eg0 = pse.tile([NBW, 2, NTW], F32, name="eg0", tag="pse")

        def mk(j):
            def f():
                pej = eg0[:, j % 2, :]
                nc.tensor.matmul(pej, KT[:, :, j * NBW:(j + 1) * NBW],
                                 QT[:, :, sl], start=True, stop=True,
                                 perf_mode=DR)
                nc.scalar.activation(et[0][:, j, :], pej, AF.Exp,
                                     scale=0.125 / 64.0)
            return f
        return [mk(j) for j in range(NB)]

    # ---- e-generation for tiles 1..3: pairs with one batched exp ----
    def eg_pair_thunks(t):
        sl = slice(t * NTW, (t + 1) * NTW)

        def mk(j):
            def f():
                pp = pse.tile([NBW, 2, NTW], F32, name="pp", tag="pse")
                nc.tensor.matmul(pp[:, 0, :], KT[:, :, j * NBW:(j + 1) * NBW],
                                 QT[:, :, sl], start=True, stop=True,
                                 perf_mode=DR)
                nc.tensor.matmul(pp[:, 1, :],
                                 KT[:, :, (j + 1) * NBW:(j + 2) * NBW],
                                 QT[:, :, sl], start=True, stop=True,
                                 perf_mode=DR)
                nc.scalar.activation(et[t][:, j:j + 2, :], pp, AF.Exp,
                                     scale=0.125 / 64.0)
            return f
        return [mk(j) for j in range(0, NB, 2)]

    def colsum_thunks(t):
        """rdbc[t][p, n] = 64 / (s[n] + colsum(e^T)[n]), all partitions."""
        sl = slice(t * NTW, (t + 1) * NTW)
        pcs = pscr.tile([NBW, NTW], F32, name="pcs", tag="scr")
        th = []
        for k in range(NP):
            def f(k=k):
                nc.tensor.matmul(pcs, ones8, et[t][:, 2 * k:2 * k + 2, :],
                                 start=(k == 0), stop=False, perf_mode=DR)
            th.append(f)
        th.append(lambda: nc.tensor.matmul(
            pcs, ones8[0:1, 0, :], s8row[0:1, sl], start=False, stop=True))
        th.append(lambda: nc.vector.reciprocal(rdbc[t], pcs))
        return th

    # ---- phase-1 Y tiles: Y[m,b] = X_b @ [8*k_r[m]|8*k_u[m]] ----
    # stored fp8 [node%128, node//128, m, u']; copies split DVE/ACT
    y = [ypool.tile([NBW, NB, 3, 2 * U], FP8, name=f"y_{b}", tag=f"y{b}")
         for b in range(BL)]

    # drain-engine assignment per batch (even-j, odd-j): chosen so each
    # prologue engine's queue stays under the ~17us exp-bound window, and
    # ACT only gets drains whose emission lands after the exp stream.
    Y_ENG = {0: ("dve", "dve"), 1: ("dve", "act"),
             2: ("dve", "dve"), 3: ("dve", "act")}

    def ygen_thunks(b):
        # batches 0-2 run in the prologue, where the psacc banks are still
        # idle: alternating psacc/pscr gives a 6-deep scratch rotation so
        # lagging drains never head-block the PE stream.
        def mk(j):
            def f():
                nsl = slice(j * NBW, (j + 1) * NBW)
                if b < 3 and j % 2 == 1:
                    py = psacc.tile([NBW, 3 * 2 * U], F32, name="py",
                                    tag="acc")
                else:
                    py = pscr.tile([NBW, 3 * 2 * U], F32, name="py",
                                   tag="scr")
                nc.tensor.matmul(py, xT[b][:, nsl], kkall, start=True,
                                 stop=True)
                eng = Y_ENG[b][j % 2]
                src = py.rearrange("p (m u) -> p m u", m=3)
                if eng == "dve":
                    nc.vector.tensor_copy(y[b][:, j, :, :], src)
                else:
                    nc.scalar.activation(
                        y[b][:, j, :, :].rearrange("p m u -> p (m u)"),
                        py, AF.Copy)
            return f
        return [mk(j) for j in range(NB)]

    def interleave(main, extra, ratio=2):
        mi = ei = 0
        while mi < len(main) or ei < len(extra):
            for _ in range(ratio):
                if mi < len(main):
                    main[mi](); mi += 1
            if ei < len(extra):
                extra[ei](); ei += 1

    # =================== phase 1: r & u gates ===================
    # Each gate group: e-support contraction first (own PSUM group), then a
    # DVE multiply by rdbc[t] on the PSUM partial, then identity + adjacency
    # terms accumulate on top (start=False).
    def e_thunks(yt, t, pa):
        th = []
        for k in range(NP):
            def f(k=k):
                nc.tensor.matmul(pa, yt[:, 2 * k:2 * k + 2, 2, :],
                                 et[t][:, 2 * k:2 * k + 2, :],
                                 start=(k == 0), stop=(k == NP - 1),
                                 perf_mode=DR)
            th.append(f)
        return th

    def rest_thunks1(b, t, sl, a1, a2, pa):
        th = [lambda: nc.vector.tensor_mul(pa, pa, rdbc[t]),
              lambda: nc.tensor.matmul(pa, kk0, xT[b][:, sl], start=False,
                                       stop=False, skip_group_check=True)]
        for m, mov in ((0, a1), (1, a2)):
            for k in range(NP):
                def f(m=m, mov=mov, k=k):
                    nc.tensor.matmul(pa, y[b][:, 2 * k:2 * k + 2, m, :],
                                     mov[:, 2 * k:2 * k + 2, :],
                                     start=False,
                                     stop=(m == 1 and k == NP - 1),
                                     perf_mode=DR, skip_group_check=True)
                th.append(f)
        return th

    def gate1(b, t, sl, pa):
        def f():
            # sigmoid(z) computed as 0.5 + 0.5*tanh(z/2): keeps every ACT
            # function in the exp_and_others table (no table reloads).
            # sig holds T = tanh(z/2) = 2*sigmoid(z) - 1.
            sig = stage.tile([128, NTW], F32, name="sig", tag="sig")
            nc.scalar.activation(sig, pa, AF.Tanh, scale=GATE_SCALE / 2,
                                 bias=bruh)
            # rows 0:64 in place: (T_r + 1)*h = 2*r*h; the c-pass kernels'
            # h-rows carry a compensating 1/2. u kept as raw T_u.
            rtmp = stage.tile([U, NTW], F32, name="rtmp", tag="rtmp")
            nc.gpsimd.tensor_add(rtmp, sig[0:U, :], one_bc[0:U, :])
            nc.gpsimd.tensor_mul(xT[b][0:U, sl], rtmp, xT[b][0:U, sl])
            dst = usig[t][b // 2][(b % 2) * U:(b % 2 + 1) * U, :]
            if b % 2 == 0:
                # cross-partition move (rows 64:128 -> 0:64): DVE only
                nc.vector.tensor_copy(dst, sig[U:128, :])
            else:
                nc.gpsimd.tensor_copy(dst, sig[U:128, :])
        return [f]

    # u stays resident in SBUF (T_u rows per (tile, batch-pair)): no
    # uscr DRAM round-trip, no phase-2 up prefetch DMAs
    usig = [[persist.tile([128, NTW], F32, name=f"usig{t}_{p}",
                          tag=f"us{t}{p}") for p in range(2)]
            for t in range(NT)]

    yc = [None, None]

    def ycgen_half(yct, p, half):
        b = 2 * p + half
        usl = slice(half * U, (half + 1) * U)
        th = []
        for j in range(NB):
            def f(b=b, usl=usl, j=j, yct=yct, half=half):
                nsl = slice(j * NBW, (j + 1) * NBW)
                pyc = pscr.tile([NBW, 3 * U], F32, name="pyc", tag="scr")
                nc.tensor.matmul(pyc, xT[b][:, nsl], kcall,
                                 start=True, stop=True)
                dst = yct[:, j, :, usl]
                src = pyc.rearrange("p (m u) -> p m u", m=3)
                if (half * NB + j) % 2 == 0:
                    nc.vector.tensor_copy(dst, src)
                else:
                    nc.scalar.activation(dst, src, AF.Copy)
            th.append(f)
        return th

    def ycgen_thunks(yct, p):
        return ycgen_half(yct, p, 0) + ycgen_half(yct, p, 1)

    # ---------- prologue ----------
    # Each QK-prelude tile is chased immediately by its four tile-0 QK
    # blocks (subtile deps let eg0[j] start as soon as its K block lands),
    # with the colsum chasing each exp pair and y-generation as PE filler.
    # The serial exp stream on ACT is tile-0's critical path; everything
    # else here is arranged to keep it back-to-back.
    qk = []
    for t in range(NT):
        qk += qk_thunks(t)
    interleave(qk, ygen_thunks(0), ratio=1)
    eg0 = eg0_thunks()
    cs0 = colsum_thunks(0)
    crit = []
    for k in range(NP):
        crit += [eg0[2 * k], eg0[2 * k + 1], cs0[k]]
    crit += cs0[NP:]
    sth = []
    for t in range(NT):
        sth += s_thunks(t)
    filler = []
    yg1 = ygen_thunks(1)
    for i, f in enumerate(yg1):
        filler.append(f)
        if i % 2 == 1 and sth:
            filler.append(sth.pop(0))
    filler += sth
    interleave(filler, crit, ratio=2)
    # tile-1's QK pairs ride the late prologue (their pse slots free as
    # tile-0's exps complete; their exps follow et0's on ACT) so rdbc[1]
    # is ready well before tile 1 needs it
    interleave(ygen_thunks(2), eg_pair_thunks(1), ratio=2)

    for t in range(NT):
        sl = slice(t * NTW, (t + 1) * NTW)
        a1 = adjslice("a1T", t)
        a2 = adjslice("a2T", t)
        pa = [psacc.tile([128, NTW], F32, name="pa", tag="acc")
              for _ in range(BL)]
        # schedule: e-streams lead their group's rest by one slot so the
        # DVE rdbc-multiply hides under the next group's e-stream.
        main = []
        main += e_thunks(y[0], t, pa[0]) + e_thunks(y[1], t, pa[1])
        if t == 0:
            # batch-3 y-generation woven into the rdbc-gated stretch
            m1 = e_thunks(y[2], t, pa[2]) \
                + rest_thunks1(0, t, sl, a1, a2, pa[0]) \
                + gate1(0, t, sl, pa[0])
            mi = []
            yg3 = ygen_thunks(3)
            k = 0
            for i, f in enumerate(m1):
                mi.append(f)
                while k < len(yg3) * (i + 1) // len(m1):
                    mi.append(yg3[k]); k += 1
            mi.extend(yg3[k:])
            main += mi
        else:
            main += e_thunks(y[2], t, pa[2])
            main += rest_thunks1(0, t, sl, a1, a2, pa[0]) \
                + gate1(0, t, sl, pa[0])
        m2 = e_thunks(y[3], t, pa[3]) \
            + rest_thunks1(1, t, sl, a1, a2, pa[1]) + gate1(1, t, sl, pa[1])
        post = []
        post += rest_thunks1(2, t, sl, a1, a2, pa[2]) + gate1(2, t, sl, pa[2])
        post += rest_thunks1(3, t, sl, a1, a2, pa[3]) + gate1(3, t, sl, pa[3])
        extra = eg_pair_thunks(t + 1) if 0 < t < NT - 1 else []
        if t == NT - 1:
            # yc[0] reads post-rh xT rows: batch 0's half can weave as soon
            # as gate1(0) has run, batch 1's into the post stream.
            yc[0] = ypool.tile([NBW, NB, 3, 2 * U], FP8, name="yc_0",
                               tag="y0")
            interleave(main, [])
            interleave(m2, ycgen_half(yc[0], 0, 0), ratio=1)
            interleave(post, ycgen_half(yc[0], 0, 1), ratio=1)
        else:
            allm = main + m2 + post
            alle = extra
            interleave(allm, alle, ratio=max(1, len(allm) // max(1, len(alle))))
        # colsum for t+1 at tile end: its exp deps had the whole tile to run
        if t + 1 < NT:
            for f in colsum_thunks(t + 1):
                f()

    # =================== phase 2+3: c gate & h_new ===================
    yc[1] = ypool.tile([NBW, NB, 3, 2 * U], FP8, name="yc_1", tag="y1")
    a1 = adjslice("a1T", 0)
    a2 = adjslice("a2T", 0)

    def hu_prefetch(p, t, sl, hu):
        # single 2-batch DMA each for h and u: [2, 64, w] -> [128, w]
        b0 = 2 * p

        def f():
            hp = p3p.tile([128, NTW], F32, name="hp", tag="hp")
            nc.sync.dma_start(
                out=hp, in_=d["hT"][b0:b0 + 2, :, sl].rearrange(
                    "b u w -> (b u) w"))
            hu.extend((hp, usig[t][p]))
        return [f]

    def rest_thunks2(p, t, sl, a1, a2, pa, hu):
        b0, b1 = 2 * p, 2 * p + 1

        th = [lambda: nc.vector.tensor_mul(pa, pa, rdbc[t]),
              lambda: nc.tensor.matmul(pa[0:U, :], kc0, xT[b0][:, sl],
                                       start=False, stop=False,
                                       skip_group_check=True),
              lambda: nc.tensor.matmul(pa[U:128, :], kc0, xT[b1][:, sl],
                                       start=False, stop=False,
                                       skip_group_check=True)]
        for m, mov in ((0, a1), (1, a2)):
            for k in range(NP):
                def f(m=m, mov=mov, k=k):
                    nc.tensor.matmul(pa, yc[p][:, 2 * k:2 * k + 2, m, :],
                                     mov[:, 2 * k:2 * k + 2, :],
                                     start=False,
                                     stop=(m == 1 and k == NP - 1),
                                     perf_mode=DR, skip_group_check=True)
                th.append(f)
        return th

    def tail2(p, t, sl, pa, hu, splits=2):
        b0, b1 = 2 * p, 2 * p + 1

        def f():
            hp, up = hu
            ct = stage.tile([128, NTW], F32, name="ct", tag="sig")
            t1 = p3p.tile([128, NTW], F32, name="t1", tag="t1")
            # compute in column chunks so ACT/Pool/DVE stages pipeline, but
            # write out full-width (HWDGE descriptor slots are the scarce
            # resource at the drain)
            for c0 in range(0, NTW, NTW // splits):
                cs = slice(c0, c0 + NTW // splits)
                nc.scalar.activation(ct[:, cs], pa[:, cs], AF.Tanh,
                                     scale=GATE_SCALE, bias=bc2)
                # h_new = c + u*(h-c) with up = T_u = 2u-1:
                # t1 = h-c (Pool); t1 = (T_u+1)*t1; t1 = t1/2 + c (DVE)
                nc.gpsimd.tensor_sub(t1[:, cs], hp[:, cs], ct[:, cs])
                nc.vector.scalar_tensor_tensor(
                    t1[:, cs], up[:, cs], 1.0, t1[:, cs],
                    mybir.AluOpType.add, mybir.AluOpType.mult)
                nc.vector.scalar_tensor_tensor(
                    t1[:, cs], t1[:, cs], 0.5, ct[:, cs],
                    mybir.AluOpType.mult, mybir.AluOpType.add)
                if splits == 4 and c0 + NTW // splits in (NTW // 2, NTW):
                    hs = slice(c0 + NTW // splits - NTW // 2,
                               c0 + NTW // splits)
                    # the two final half-writes issue on different queues
                    # so their HWDGE/issue pipelines overlap
                    eng = nc.sync if hs.start == 0 else nc.scalar
                    eng.dma_start(
                        out=out_h[b0:b0 + 2, :,
                                  t * NTW + hs.start:t * NTW + hs.stop]
                        .rearrange("b u w -> (b u) w"),
                        in_=t1[:, hs])
            if splits != 4:
                nc.sync.dma_start(
                    out=out_h[b0:b0 + 2, :, sl].rearrange("b u w -> (b u) w"),
                    in_=t1)
        return [f]

    prev = []
    pa_pre = pa_pre1 = None
    for t in range(NT):
        sl = slice(t * NTW, (t + 1) * NTW)
        if t > 0:
            a1 = adjslice("a1T", t)
            a2 = adjslice("a2T", t)
        pa0 = pa_pre if pa_pre is not None else \
            psacc.tile([128, NTW], F32, name="pa2", tag="acc")
        pa1 = pa_pre1 if pa_pre1 is not None else \
            psacc.tile([128, NTW], F32, name="pa2", tag="acc")
        hu0, hu1 = [], []
        main = hu_prefetch(0, t, sl, hu0) + prev + hu_prefetch(1, t, sl, hu1)
        if t == 0:
            # p0's whole contraction only needs yc[0] (built at the end of
            # phase 1): run it as the dense stream with yc[1] generation
            # woven in as drain-paced filler, then chase e(yc[1]).
            p0w = e_thunks(yc[0], t, pa0) \
                + rest_thunks2(0, t, sl, a1, a2, pa0, hu0)
            ycg = ycgen_thunks(yc[1], 1)
            mi = []
            k = 0
            for i, f in enumerate(p0w):
                mi.append(f)
                while k < len(ycg) * (i + 1) // len(p0w):
                    mi.append(ycg[k]); k += 1
            main += mi + ycg[k:] + e_thunks(yc[1], t, pa1)
        else:
            if pa_pre is None:
                main += e_thunks(yc[0], t, pa0)
            if pa_pre1 is None:
                main += e_thunks(yc[1], t, pa1)
            main += rest_thunks2(0, t, sl, a1, a2, pa0, hu0)
        # pre-stream the next tile's e-contractions: PE filler while this
        # tile's tanh/tail chains drain (p1's bank alternates between the
        # phase-2-idle pse pool and a dedicated pscr tag)
        if t + 1 < NT:
            pa_pre = psacc.tile([128, NTW], F32, name="pa2", tag="acc")
            main += e_thunks(yc[0], t + 1, pa_pre)
            if t % 2 == 0:
                pa_pre1 = pse.tile([128, NTW], F32, name="pa2e", tag="pse")
            else:
                pa_pre1 = psacc.tile([128, NTW], F32, name="pa2e", tag="acc")
            main += e_thunks(yc[1], t + 1, pa_pre1)
        else:
            pa_pre = pa_pre1 = None
        main += tail2(0, t, sl, pa0, hu0)
        for f in main:
            f()
        # carry p=1's rest into the next tile so its DVE multiply and the
        # tail chain overlap with the next tile's e-streams
        prev = rest_thunks2(1, t, sl, a1, a2, pa1, hu1) \
            + tail2(1, t, sl, pa1, hu1, splits=4 if t == NT - 1 else 2)
    for f in prev:
        f()

    ctx.close()


_CACHE = {}


def _get_program():
    if "nc" not in _CACHE:
        _CACHE["nc"] = _build_program()
    return _CACHE["nc"]


def _prep_inputs(inputs, h_prev, adj1, adj2, feat, SE, Wq, Wk, Ws1, bs1, Ws2,
                 bs2, r_kernel, r_bias, u_kernel, u_bias, c_kernel, c_bias):
    bf = ml_dtypes.bfloat16
    f8 = ml_dtypes.float8_e4m3fn
    f32 = np.float32
    perm = list(range(DIN, FROWS)) + list(range(DIN))  # [h(64); inputs(2)]

    h3 = np.asarray(h_prev, f32).reshape(B, N, U)
    hT = np.ascontiguousarray(h3.transpose(0, 2, 1))            # [B, U, N]
    inT = np.asarray(inputs, f32).transpose(0, 2, 1)            # [B, DIN, N]
    xT = np.concatenate([hT, inT], axis=1).astype(bf)           # [B, 66, N]

    rk = np.asarray(r_kernel, f32)[:, perm, :]
    uk = np.asarray(u_kernel, f32)[:, perm, :]
    ck = np.asarray(c_kernel, f32)[:, perm, :]
    kkall = np.concatenate(
        [np.concatenate([rk[m], uk[m]], axis=1) for m in (1, 2, 3)],
        axis=1) * SC_Y                                          # [66, 384]
    kk0 = np.concatenate([rk[0], uk[0]], axis=1) * SC_ID        # [66, 128]
    # c-pass x_cat holds 2*r*h in its h-rows (r comes from the tanh-form
    # sigmoid as T_r + 1); compensate with a 1/2 on the kernels' h-rows.
    ck = ck.copy()
    ck[:, 0:U, :] *= 0.5
    kcall = np.concatenate([ck[1], ck[2], ck[3]], axis=1) * SC_Y
    kc0 = ck[0] * SC_ID

    wpk = np.zeros((FD + SD, 961), f32)
    wpk[0:FD, 0:64] = np.asarray(Wq, f32)
    wpk[0:FD, 64:128] = np.asarray(Wk, f32)
    wpk[:, 128:192] = np.asarray(Ws1, f32)
    wpk[0:FROWS, 192:576] = kkall
    wpk[0:FROWS, 576:704] = kk0
    wpk[0:FROWS, 704:896] = kcall
    wpk[0:FROWS, 896:960] = kc0
    wpk[0:U, 960] = np.asarray(Ws2, f32).reshape(U)
    bpk = np.zeros((2 * U, 4), f32)
    bpk[:, 0] = np.concatenate([np.asarray(r_bias, f32).mean(0),
                                np.asarray(u_bias, f32).mean(0)]) * 0.5
    bpk[:, 1] = np.tile(np.asarray(c_bias, f32).mean(0), 2)
    bpk[0:U, 2] = np.asarray(bs1, f32)
    bpk[0, 3] = np.asarray(bs2, f32).reshape(1)[0]

    shared = {
        "a1T": np.ascontiguousarray(np.asarray(adj1, f32).T * SC_ADJ).astype(f8),
        "a2T": np.ascontiguousarray(np.asarray(adj2, f32).T * SC_ADJ).astype(f8),
        "fsT": np.ascontiguousarray(
            np.concatenate([np.asarray(feat, f32).T, np.asarray(SE, f32).T],
                           axis=0)).astype(bf),
        "wpack": wpk.astype(bf),
        "bpack": bpk,
        "ones8": np.full((NBW, 2, NBW), 1.0 / 64.0, f8),
    }
    in_maps = []
    for c in range(NCORES):
        bsl = slice(c * BL, (c + 1) * BL)
        m = dict(shared)
        m["xT"] = np.ascontiguousarray(xT[bsl])
        m["hT"] = np.ascontiguousarray(hT[bsl])
        in_maps.append(m)
    return in_maps


def kernel(**inputs):
    os.environ.setdefault("NEURON_RT_RESET_CORES", "1")
    nc = _get_program()
    in_maps = _prep_inputs(**inputs)
    res = None
    err = None
    for _ in range(2):
        try:
            res = run_bass_kernel_spmd(nc, in_maps, list(range(NCORES)))
            break
        except Exception as e:  # e.g. a wedged device; retry once
            err = e
    if res is None:
        raise err
    outs = []
    for c in range(NCORES):
        o = res.results[c]["out"]                     # [BL, U, N] f32
        outs.append(o.transpose(0, 2, 1).reshape(BL, N * U))
    return np.concatenate(outs, axis=0).astype(np.float32)
